# revision 1
# baseline (speedup 1.0000x reference)
"""Multi-head attention (B=2, L=2048, D=2048, H=16, causal + RoPE) on 8 TRN2 cores.

Sharding: tensor-parallel over heads. Core c owns heads {2c, 2c+1}:
  - wq/wk/wv column slices [D, 256], wo row slice [256, D]
  - each core computes a partial output y_c = att_c @ wo_c  (full shape)
  - host reduces: y = sum_c y_c   (the "all-reduce" of the output projection)

v2 schedule (vs baseline):
  - Projections and attention WOVEN: attention i-tile t (512 queries) only
    needs token-chunks <= 2t+1 of Q/K/V, so the remaining projection chains
    interleave with attention j-steps, keeping the PE dense end-to-end.
  - Softmax normalization: denominators r [1,512] inverted on DVE,
    partition-broadcast on GPSIMD, folded into the AV PSUM->SBUF drain.
    att in SBUF is already normalized; no DRAM transpose bounce.
  - y tiles: both heads' matmuls chained into ONE PSUM bank, drained by a
    single copy alternating ACT/DVE; trickled into attention j-steps and
    the next batch's projection phase.
  - Diagonal score blocks are column-restricted (masked columns i < 128q
    not computed) where the moving dim stays >= 256.
  - x chunks prefetched 2 chunks ahead on the sync DMA queue; first chunk
    interleaved with wq so the first chain starts early.
"""

import glob
import os


def _ensure_env():
    # walrus_driver (neuronx-cc) must be on PATH for client-side NEFF compile.
    if not any("-b16-bazel-" in p for p in os.environ.get("PATH", "").split(":")):
        cands = sorted(glob.glob("/nix/store/*-b16-bazel-*/bin"))
        for c in cands:
            if os.path.exists(os.path.join(c, "neuronx-cc")) or glob.glob(
                os.path.join(c, "*walrus*")
            ):
                os.environ["PATH"] = c + ":" + os.environ["PATH"]
                break
        else:
            if cands:
                os.environ["PATH"] = cands[-1] + ":" + os.environ["PATH"]


_ensure_env()
os.environ.setdefault("JAX_COMPILATION_CACHE_DIR", "/tmp/jax_comp_cache")
os.environ.setdefault("JAX_PERSISTENT_CACHE_MIN_COMPILE_TIME_SECS", "1")
os.environ.setdefault("JAX_PERSISTENT_CACHE_MIN_ENTRY_SIZE_BYTES", "0")

import numpy as np  # noqa: E402

import concourse.bass as bass  # noqa: E402
import concourse.mybir as mybir  # noqa: E402
import concourse.tile as tile  # noqa: E402
from concourse import bacc  # noqa: E402
from concourse.bass_utils import run_bass_kernel_spmd  # noqa: E402

NCORES = 8
B, L, D = 2, 2048, 2048
H = 16
HD = 128            # head dim
HPC = H // NCORES   # heads per core
DQ = HPC * HD       # 256: per-core projection width
ROPE = 64           # RoPE dims per head
F32 = mybir.dt.float32
F32R = mybir.dt.float32r
BF16 = mybir.dt.bfloat16

# data-path dtype for SBUF tensors feeding the PE (flip to BF16 for the
# half-width data path; fp32r keeps baseline numerics)
DT = F32R

NT256 = L // 256    # 8 token chunks per batch for projections
NCHUNK = B * NT256  # 16 chunks in the linear stream
NI = L // 512       # 4 i-tiles per attention instance
NJ = L // 128       # 16 j-blocks


def build_nc():
    nc = bacc.Bacc(
        "TRN2", target_bir_lowering=False, debug=False, num_devices=NCORES
    )
    xt = nc.dram_tensor("xt", [B, D, L], DT, kind="ExternalInput").ap()
    wq = nc.dram_tensor("wq", [D, DQ], BF16, kind="ExternalInput").ap()
    wk = nc.dram_tensor("wk", [D, DQ], BF16, kind="ExternalInput").ap()
    wv = nc.dram_tensor("wv", [D, DQ], BF16, kind="ExternalInput").ap()
    wo = nc.dram_tensor("wo", [DQ, D], DT, kind="ExternalInput").ap()
    xbf = nc.dram_tensor("xbf", [D, 512], BF16, kind="ExternalInput").ap()
    # cc rows = [cos;cos], ss rows = [-sin;+sin] (bf16), for pairs (i, i+32)
    cc = nc.dram_tensor("cc", [ROPE, L], BF16, kind="ExternalInput").ap()
    ss = nc.dram_tensor("ss", [ROPE, L], BF16, kind="ExternalInput").ap()
    m4 = nc.dram_tensor("m4", [128, 4, 512], BF16, kind="ExternalInput").ap()
    y = nc.dram_tensor("y", [B, L, D], BF16, kind="ExternalOutput").ap()

    with tile.TileContext(nc) as tc:
        with (
            tc.tile_pool(name="consts", bufs=1) as consts,
            tc.tile_pool(name="wpool", bufs=1) as wpool,
            tc.tile_pool(name="qkv", bufs=1) as qkv,
            tc.tile_pool(name="xc", bufs=3) as xcpool,
            tc.tile_pool(name="et", bufs=3) as etpool,
            tc.tile_pool(name="rope", bufs=2) as ropepool,
            tc.tile_pool(name="ysb", bufs=4) as ypool,
            tc.tile_pool(name="riv", bufs=1) as rivpool,
            tc.tile_pool(name="rbc", bufs=1) as rbcpool,
            tc.tile_pool(name="pA", bufs=3, space="PSUM") as pA,
            tc.tile_pool(name="pST", bufs=2, space="PSUM") as pST,
            tc.tile_pool(name="pAV", bufs=2, space="PSUM") as pAV,
            tc.tile_pool(name="pR", bufs=1, space="PSUM") as pR,
        ):
            # ---- constants ----
            ones_f = consts.tile([128, 1], F32)
            nc.vector.memset(ones_f, 1.0)
            ones = consts.tile([128, 1], DT)
            nc.vector.tensor_copy(ones, ones_f)
            zer_f = consts.tile([128, 128], F32)
            nc.vector.memset(zer_f, 0.0)
            zer128 = consts.tile([128, 128], DT)
            nc.vector.tensor_copy(zer128, zer_f)
            cc_sb = consts.tile([ROPE, L], BF16)
            nc.sync.dma_start(out=cc_sb, in_=cc)
            ss_sb = consts.tile([ROPE, L], BF16)
            nc.sync.dma_start(out=ss_sb, in_=ss)
            m4_sb = consts.tile([128, 4, 512], BF16)
            nc.sync.dma_start(out=m4_sb, in_=m4)

            wq_sb = wpool.tile([128, 16, DQ], DT)
            wk_sb = wpool.tile([128, 16, DQ], DT)
            wv_sb = wpool.tile([128, 16, DQ], DT)
            wo_sb = wpool.tile([128, HPC, D], DT)

            qt_sb = qkv.tile([128, HPC, L], DT)   # [d, h, tok]
            kt_sb = qkv.tile([128, HPC, L], DT)
            v_sb = qkv.tile([128, NJ, DQ], DT)    # [tok_in_blk, jblk, hd]
            att_sb = qkv.tile([128, HPC, L], DT)  # [hd, h, tok] NORMALIZED

            # ---- x chunk stream (prefetched 2 chunks ahead) ----
            xc_tiles = {}

            def _stage(t_, oct_):
                # bf16 oct lands in the upper half of the f32r tile
                return t_.bitcast(BF16)[:, 8 + oct_, :]

            def _widen_src(t_):
                return t_.bitcast(BF16)[:, 8:16, :].rearrange(
                    "p a t -> p (a t)"
                ).rearrange("p (c o) -> p c o", c=16)

            def emit_xc_dma(i):
                if i >= NCHUNK or i in xc_tiles:
                    return
                b_, tt_ = divmod(i, NT256)
                t_ = xcpool.tile([128, 16, 256], DT, tag="xc", name=f"xc{i}")
                xc_tiles[i] = t_
                for oct_ in range(8):
                    nc.sync.dma_start(
                        out=t_[:, 2 * oct_ : 2 * oct_ + 2, :],
                        in_=xt[
                            b_,
                            256 * oct_ : 256 * oct_ + 256,
                            256 * tt_ : 256 * tt_ + 256,
                        ].rearrange("(c p) t -> p c t", p=128),
                    )

            # priming: wq octs interleaved with x chunk 0, then x chunk 1,
            # wk, wv (wo is emitted later, during the prologue)
            def _stage_w(w_dram, oct_):
                # stage bf16 weight octs in ysb tiles (bf16, never an FP32r
                # matmul input, unused until the first y-emit ~45us in), so
                # the BIR verifier's rounded-to-FP32r rule is satisfied: the
                # weight tile itself is only ever written by the DVE widen
                pstg = ypool.tile([128, 512], BF16, tag="ysb", name="wstg")
                nc.sync.dma_start(
                    out=pstg.rearrange("p (c o) -> p c o", c=2),
                    in_=w_dram[256 * oct_ : 256 * oct_ + 256, :].rearrange(
                        "(c p) o -> p c o", p=128
                    ),
                )
                return pstg

            def _stage_x(t_, b_, oct_, tsl):
                nc.sync.dma_start(
                    out=t_[:, 2 * oct_ : 2 * oct_ + 2, :],
                    in_=xt[b_, 256 * oct_ : 256 * oct_ + 256, tsl].rearrange(
                        "(c p) t -> p c t", p=128
                    ),
                )

            def _widen_oct(w_sb, pstg, oct_):
                nc.vector.tensor_copy(
                    w_sb[:, 2 * oct_ : 2 * oct_ + 2, :],
                    pstg.rearrange("p (c o) -> p c o", c=2),
                )

            xstage = xcpool.tile([128, 16, 256], DT, tag="xc", name="xstage")

            def _stage_xbf(oct_, half):
                v = xstage.bitcast(BF16)[:, 8 * half + oct_, :]
                nc.sync.dma_start(
                    out=v.rearrange("p (c t) -> p c t", c=2),
                    in_=xbf[
                        256 * oct_ : 256 * oct_ + 256,
                        256 * half : 256 * half + 256,
                    ].rearrange("(c p) t -> p c t", p=128),
                )

            def _widen_xbf(t_, oct_, half):
                v = xstage.bitcast(BF16)[:, 8 * half + oct_, :]
                nc.scalar.activation(
                    t_[:, 2 * oct_ : 2 * oct_ + 2, :],
                    v.rearrange("p (c t) -> p c t", c=2),
                    mybir.ActivationFunctionType.Copy,
                )

            xc_pre = xcpool.tile([128, 16, 256], DT, tag="xc", name="xc0")
            xc_tiles[0] = xc_pre
            for oct_ in range(8):
                pq = _stage_w(wq, oct_)
                _stage_xbf(oct_, 0)
                _widen_oct(wq_sb, pq, oct_)
                _widen_xbf(xc_pre, oct_, 0)
            # wk octs interleaved with x chunk 1: QT(tt1) and KT(tt0)
            # both unblock progressively
            xc1 = xcpool.tile([128, 16, 256], DT, tag="xc", name="xc1")
            xc_tiles[1] = xc1
            for oct_ in range(8):
                pk = _stage_w(wk, oct_)
                _stage_xbf(oct_, 1)
                _widen_oct(wk_sb, pk, oct_)
                _widen_xbf(xc1, oct_, 1)
            for oct_ in range(8):
                pv_ = _stage_w(wv, oct_)
                _widen_oct(wv_sb, pv_, oct_)

            def unit_wo(half):
                def run():
                    for h in range(HPC):
                        nc.sync.dma_start(
                            out=wo_sb[:, h, 1024 * half : 1024 * half + 1024],
                            in_=wo[
                                HD * h : HD * h + HD,
                                1024 * half : 1024 * half + 1024,
                            ],
                        )
                return run

            # ---- y emission ----
            pending = []   # (b, t2, dd)
            ycnt = [0]

            def emit_y(b, t2, dd):
                p = pA.tile([128, 512], F32, tag="pA", name=f"yp_{b}_{t2}_{dd}")
                nc.tensor.matmul(
                    p,
                    (att_sb[:, 0, 128 * t2 : 128 * t2 + 128]),
                    (wo_sb[:, 0, 512 * dd : 512 * dd + 512]),
                    start=True,
                    stop=False,
                )
                nc.tensor.matmul(
                    p,
                    (att_sb[:, 1, 128 * t2 : 128 * t2 + 128]),
                    (wo_sb[:, 1, 512 * dd : 512 * dd + 512]),
                    start=False,
                    stop=True,
                )
                yt = ypool.tile([128, 512], BF16, tag="ysb")
                if ycnt[0] % 2 == 0:
                    nc.scalar.activation(
                        yt, p, mybir.ActivationFunctionType.Copy
                    )
                else:
                    nc.vector.tensor_copy(yt, p)
                ycnt[0] += 1
                nc.gpsimd.dma_start(
                    out=y[
                        b,
                        128 * t2 : 128 * t2 + 128,
                        512 * dd : 512 * dd + 512,
                    ],
                    in_=yt,
                )

            def pop_y():
                if pending:
                    emit_y(*pending.pop(0))

            # ---- projection units ----
            def unit_qk_chain(i, w_sb, out_sb, rt, drain_act):
                def run():
                    xc = xc_tiles[i]
                    tt = i % NT256
                    pp = pA.tile([128, 512], F32, tag="pA")
                    pj = pp[:, 0:256]
                    for c in range(16):
                        nc.tensor.matmul(
                            pj,
                            (w_sb[:, c, 128 * rt : 128 * rt + 128]),
                            (xc[:, c, :]),
                            start=(c == 0),
                            stop=(c == 15),
                        )
                    dst = out_sb[:, rt, 256 * tt : 256 * tt + 256]
                    if drain_act:
                        nc.scalar.activation(
                            dst, pj, mybir.ActivationFunctionType.Copy
                        )
                    else:
                        nc.vector.tensor_copy(dst, pj)
                return run

            def unit_v_chain(i, ts2):
                def run():
                    xc = xc_tiles[i]
                    tt = i % NT256
                    pv = pA.tile([128, 512], F32, tag="pA")
                    pvj = pv[:, 0:256]
                    for c in range(16):
                        nc.tensor.matmul(
                            pvj,
                            (xc[:, c, 128 * ts2 : 128 * ts2 + 128]),
                            (wv_sb[:, c, :]),
                            start=(c == 0),
                            stop=(c == 15),
                        )
                    nc.vector.tensor_copy(v_sb[:, 2 * tt + ts2, :], pvj)
                return run

            def unit_rope(out_sb, rt, qq, dq):
                # rot = [x1;x2]*[c;c] + [x2;x1]*[-s;s] on the 512-tok quarter
                # swap DMAs ride the ACT/DVE DGE queues so they never block
                # the x-chunk prefetches on the sync queue
                def run():
                    qsl = slice(512 * qq, 512 * qq + 512)
                    rope_rows = out_sb[0:ROPE, rt, qsl]
                    swap = ropepool.tile([ROPE, 512], DT, tag="rope")
                    dq.dma_start(out=swap[0:32], in_=out_sb[32:64, rt, qsl])
                    dq.dma_start(out=swap[32:64], in_=out_sb[0:32, rt, qsl])
                    nc.vector.tensor_mul(swap, swap, ss_sb[:, qsl])
                    nc.vector.tensor_mul(rope_rows, rope_rows, cc_sb[:, qsl])
                    nc.vector.tensor_add(rope_rows, rope_rows, swap)
                return run

            def chunk_units(i):
                units = [lambda: emit_xc_dma(i + 2)]
                units.append(unit_qk_chain(i, wq_sb, qt_sb, 0, True))
                units.append(unit_qk_chain(i, wq_sb, qt_sb, 1, True))
                units.append(unit_qk_chain(i, wk_sb, kt_sb, 0, False))
                units.append(unit_qk_chain(i, wk_sb, kt_sb, 1, False))
                units.append(unit_v_chain(i, 0))
                units.append(unit_v_chain(i, 1))
                return units

            def chunk_qk_units(i):
                units = [lambda: emit_xc_dma(i + 2)]
                units.append(unit_qk_chain(i, wq_sb, qt_sb, 0, True))
                units.append(unit_qk_chain(i, wq_sb, qt_sb, 1, True))
                units.append(unit_qk_chain(i, wk_sb, kt_sb, 0, False))
                units.append(unit_qk_chain(i, wk_sb, kt_sb, 1, False))
                return units

            def chunk_v_units(i):
                return [unit_v_chain(i, 0), unit_v_chain(i, 1)]

            def rope_units(qq):
                out = []
                for rt in range(HPC):
                    out.append(unit_rope(qt_sb, rt, qq, nc.scalar))
                    out.append(unit_rope(kt_sb, rt, qq, nc.scalar))
                return out

            # ---- attention ----
            def att_instance(b, h, t, seg, last=False):
                av = pAV.tile([128, 512], F32, tag="pAV")
                rp = pR.tile([1, 512], F32, tag="pR")
                njb = 4 * t + 4
                for j in range(njb):
                    q = j - 4 * t
                    # column-restrict diagonal blocks while moving dim >= 256
                    off = 128 * q if q in (1, 2) else 0
                    st = pST.tile([128, 512], F32, tag="pST")
                    nc.tensor.matmul(
                        st[:, off:512],
                        (kt_sb[:, h, 128 * j : 128 * j + 128]),
                        (qt_sb[:, h, 512 * t + off : 512 * t + 512]),
                        start=True,
                        stop=True,
                    )
                    et = etpool.tile([128, 512], DT, tag="et")
                    if q == 3:
                        for z in range(3):
                            nc.vector.tensor_copy(
                                et[:, 128 * z : 128 * z + 128], zer128
                            )
                        eoff = 384
                    else:
                        eoff = off
                    nc.scalar.activation(
                        et[:, eoff:512],
                        st[:, eoff:512],
                        mybir.ActivationFunctionType.Exp,
                    )
                    if q >= 0:
                        # only cols [128q, 128q+128) are partially masked for
                        # q<3 (beyond them causality always holds, mask==1);
                        # q3 is unrestricted so its full width needs zeroing
                        msl = slice(128 * q, 128 * q + 128)
                        nc.vector.tensor_mul(
                            et[:, msl], et[:, msl], m4_sb[:, q, msl],
                        )
                    # fillers BEFORE rp/av: the ~4-deep OOO window parks on
                    # the exp->mask chain, so independent work must sit ahead
                    # of the parked dependents in program order
                    if seg:
                        seg.pop(0)()
                    pop_y()
                    nc.tensor.matmul(
                        rp[:, off:512], (ones), (et[:, off:512]),
                        start=(j == 0), stop=(j == njb - 1),
                    )
                    nc.tensor.matmul(
                        av[:, off:512],
                        (v_sb[:, j, HD * h : HD * h + HD]),
                        (et[:, off:512]),
                        start=(j == 0), stop=(j == njb - 1),
                    )
                # normalize while draining: att = av * bcast(1/r)
                riv = rivpool.tile([1, 512], F32, tag="riv")
                nc.vector.reciprocal(riv, rp)
                rbc = rbcpool.tile([128, 512], F32, tag="rbc")
                if last:
                    # split the normalize so the tail's first y tiles (which
                    # read 128-col slices of att) unblock after half the work
                    for hf in range(4):
                        sl = slice(128 * hf, 128 * hf + 128)
                        nc.gpsimd.partition_broadcast(rbc[:, sl], riv[:, sl])
                        nc.vector.tensor_mul(
                            att_sb[:, h, 512 * t + 128 * hf :
                                   512 * t + 128 * hf + 128],
                            av[:, sl], rbc[:, sl],
                        )
                else:
                    nc.gpsimd.partition_broadcast(rbc, riv)
                    nc.vector.tensor_mul(
                        att_sb[:, h, 512 * t : 512 * t + 512], av, rbc
                    )

            # ---------------- main schedule ----------------
            for b in range(B):
                base = NT256 * b
                emit_xc_dma(base)
                emit_xc_dma(base + 1)
                pro = chunk_units(base) + chunk_units(base + 1) + rope_units(0)
                if b == 0:
                    pro.append(unit_wo(0))
                    pro.append(unit_wo(1))
                for u in pro:
                    u()
                    pop_y()
                for t in range(NI):
                    if t < NI - 1:
                        seg = (
                            chunk_qk_units(base + 2 * t + 2)
                            + chunk_qk_units(base + 2 * t + 3)
                            + rope_units(t + 1)
                            + chunk_v_units(base + 2 * t + 2)
                            + chunk_v_units(base + 2 * t + 3)
                        )
                    else:
                        seg = []
                    for h in range(HPC):
                        att_instance(
                            b, h, t, seg,
                            last=(b == B - 1 and t == NI - 1 and h == HPC - 1),
                        )
                    while seg:
                        seg.pop(0)()
                        pop_y()
                    pending.extend(
                        (b, t2, dd)
                        for t2 in range(4 * t, 4 * t + 4)
                        for dd in range(4)
                    )
            while pending:
                emit_y(*pending.pop(0))
    nc.compile()
    return nc


_NC = None


def _get_nc():
    global _NC
    if _NC is None:
        _NC = build_nc()
    return _NC


def _np_dt():
    if DT == BF16:
        import ml_dtypes

        return ml_dtypes.bfloat16
    return np.float32


def _host_inputs(x, mask, wq, wk, wv, wo):
    x = np.asarray(x, np.float32)
    wq = np.asarray(wq, np.float32)
    wk = np.asarray(wk, np.float32)
    wv = np.asarray(wv, np.float32)
    wo = np.asarray(wo, np.float32)

    import ml_dtypes

    ndt = _np_dt()
    xt = np.ascontiguousarray(x.transpose(0, 2, 1)).astype(ndt)  # [B, D, L]

    # permute head dims so RoPE pairs are (i, i+32): [evens, odds, pass-through]
    perm128 = np.concatenate(
        [np.arange(0, ROPE, 2), np.arange(1, ROPE, 2), np.arange(ROPE, HD)]
    )
    permD = np.concatenate([h * HD + perm128 for h in range(H)])
    wq_p = (wq * np.float32(1.0 / np.sqrt(HD)))[:, permD]
    wk_p = wk[:, permD]

    # RoPE tables, matching reference fp32 math (dim=64, repeat-2 interleave)
    ts_ = np.arange(0, ROPE, 2, dtype=np.float32)
    inv = (np.float32(10000.0) ** (-ts_ / np.float32(ROPE))).astype(np.float32)
    grid = np.arange(L, dtype=np.float32)[:, None] * inv[None, :]  # [L, 32]
    cc = np.empty((ROPE, L), np.float32)
    cc[0:32] = cc[32:64] = np.cos(grid).T
    ss = np.empty((ROPE, L), np.float32)
    ss[0:32] = -np.sin(grid).T
    ss[32:64] = np.sin(grid).T
    cc = cc.astype(ml_dtypes.bfloat16)
    ss = ss.astype(ml_dtypes.bfloat16)

    # diagonal-quarter masks for ST tiles: m4[j, q, i] = mask[i, 128q + j]
    mm = np.asarray(mask[0, 0, :512, :512])
    m4 = (
        mm.T.reshape(4, 128, 512).transpose(1, 0, 2).astype(ml_dtypes.bfloat16)
    )  # [j, q, i]

    in_maps = []
    for c in range(NCORES):
        sl = slice(DQ * c, DQ * c + DQ)
        in_maps.append(
            {
                "xt": xt,
                "wq": np.ascontiguousarray(wq_p[:, sl]).astype(
                    ml_dtypes.bfloat16
                ),
                "wk": np.ascontiguousarray(wk_p[:, sl]).astype(
                    ml_dtypes.bfloat16
                ),
                "wv": np.ascontiguousarray(wv[:, sl]).astype(
                    ml_dtypes.bfloat16
                ),
                "wo": np.ascontiguousarray(wo[sl, :]).astype(ndt),
                "xbf": np.ascontiguousarray(
                    np.asarray(xt[0, :, 0:512], np.float32)
                ).astype(ml_dtypes.bfloat16),
                "cc": cc,
                "ss": ss,
                "m4": m4,
            }
        )
    return in_maps


def _reference_host(x, mask, wq, wk, wv, wo):
    """Exact-math numpy fallback (used only if the mask is not causal-tril)."""
    Hh, P = H, 64
    xx = np.asarray(x, np.float32)
    Bb, Ll, Dd = xx.shape
    K = Dd // Hh

    def rope(t):  # [b,h,s,d]
        d, s = t.shape[-1], t.shape[-2]
        ts_ = np.arange(0, d, 2, dtype=np.float32)
        inv = (np.float32(10000.0) ** (-ts_ / np.float32(d)))
        grid = np.arange(s, dtype=np.float32)[:, None] * inv[None, :]
        sin = np.repeat(np.sin(grid), 2, axis=-1)[None, None]
        cos = np.repeat(np.cos(grid), 2, axis=-1)[None, None]
        x1, x2 = t[..., ::2], t[..., 1::2]
        xs = np.stack([-x2, x1], axis=-1).reshape(t.shape)
        return t * cos + xs * sin

    def split(t):
        return t.reshape(Bb, Ll, Hh, K).transpose(0, 2, 1, 3)

    q = split(xx @ np.asarray(wq, np.float32)) / np.sqrt(K)
    q = np.concatenate([rope(q[..., :P]), q[..., P:]], axis=-1)
    k = split(xx @ np.asarray(wk, np.float32))
    k = np.concatenate([rope(k[..., :P]), k[..., P:]], axis=-1)
    v = split(xx @ np.asarray(wv, np.float32))
    s = np.einsum("bhik,bhjk->bhij", q, k)
    s = np.where(np.asarray(mask), s, np.float32(-1e8))
    s -= s.max(axis=-1, keepdims=True)
    e = np.exp(s)
    a = e / e.sum(axis=-1, keepdims=True)
    yy = np.einsum("bhij,bhjv->bhiv", a, v)
    yy = yy.transpose(0, 2, 1, 3).reshape(Bb, Ll, Dd)
    return (yy @ np.asarray(wo, np.float32)).astype(np.float32)


def kernel(**inputs):
    mask_arr = np.asarray(inputs["mask"])
    if not bool(
        (mask_arr[0, 0] == np.tril(np.ones((L, L), bool))).all()
    ):
        return _reference_host(
            inputs["x"], inputs["mask"], inputs["wq"], inputs["wk"],
            inputs["wv"], inputs["wo"],
        )
    nc = _get_nc()
    in_maps = _host_inputs(
        inputs["x"], inputs["mask"], inputs["wq"], inputs["wk"],
        inputs["wv"], inputs["wo"],
    )
    res = run_bass_kernel_spmd(nc, in_maps, core_ids=list(range(NCORES)))
    out = res.results[0]["y"].astype(np.float64)
    for c in range(1, NCORES):
        out += res.results[c]["y"]
    return out.astype(np.float32)



# revision 11
# speedup vs baseline: 1.2645x; 1.2645x over previous
"""Multi-head attention (B=2, L=2048, D=2048, H=16, causal + RoPE) on 8 TRN2 cores.

Sharding: tensor-parallel over heads. Core c owns heads {2c, 2c+1}:
  - wq/wk/wv column slices [D, 256], wo row slice [256, D]
  - each core computes a partial output y_c = att_c @ wo_c  (full shape)
  - host reduces: y = sum_c y_c   (the "all-reduce" of the output projection)

v3 schedule (vs v2 baseline):
  - QKV projections in fp8(e4m3) DoubleRow with a hi/lo residual split:
    x = (xh + xl)/SX, w = (wh + wl)/SW; x@w ~ (xh@wh + xl@wh + xh@wl)
    / (SX*SW).  3 DoubleRow chains of 8 MMs replace 16 fp32r MMs per
    256-token chain (24*128 vs 16*256 PE cycles).
  - Softmax denominators via et-as-STATIONARY matmuls: out [128i, 1]
    per 128-col slice of et (free-dim cost 1 instead of width), then a
    PE transpose of the accumulated [128,4] into [1,512], reciprocal,
    partition-broadcast, and the usual normalize-on-drain.
  - qt/kt/v/et/att all bf16: halves DVE element cost where 2x modes
    apply and lifts the fp32r >=256 moving-width restriction, so
    diagonal score blocks shrink to exact causal widths 512/384/256/128.
  - x chunks DMA'd as fp8 hi|lo packed [128,16,512] tiles (2 DMAs per
    chunk instead of 8, half the bytes of f32).
  - y written back per 128-token block [128, 2048] (1 DMA instead of 4).
  - PE warm-up: a few dummy matmuls at t~0 so the p-state ramp finishes
    before the first real projection chain.
"""

import glob
import os


def _ensure_env():
    # walrus_driver (neuronx-cc) must be on PATH for client-side NEFF compile.
    if not any("-b16-bazel-" in p for p in os.environ.get("PATH", "").split(":")):
        cands = sorted(glob.glob("/nix/store/*-b16-bazel-*/bin"))
        for c in cands:
            if os.path.exists(os.path.join(c, "neuronx-cc")) or glob.glob(
                os.path.join(c, "*walrus*")
            ):
                os.environ["PATH"] = c + ":" + os.environ["PATH"]
                break
        else:
            if cands:
                os.environ["PATH"] = cands[-1] + ":" + os.environ["PATH"]


_ensure_env()
os.environ.setdefault("JAX_COMPILATION_CACHE_DIR", "/tmp/jax_comp_cache")
os.environ.setdefault("JAX_PERSISTENT_CACHE_MIN_COMPILE_TIME_SECS", "1")
os.environ.setdefault("JAX_PERSISTENT_CACHE_MIN_ENTRY_SIZE_BYTES", "0")

import numpy as np  # noqa: E402

import concourse.bass as bass  # noqa: E402
import concourse.mybir as mybir  # noqa: E402
import concourse.tile as tile  # noqa: E402
from concourse import bacc, masks  # noqa: E402
from concourse.bass_utils import run_bass_kernel_spmd  # noqa: E402

NCORES = 8
B, L, D = 2, 2048, 2048
H = 16
HD = 128            # head dim
HPC = H // NCORES   # heads per core
DQ = HPC * HD       # 256: per-core projection width
ROPE = 64           # RoPE dims per head
F32 = mybir.dt.float32
F32R = mybir.dt.float32r
BF16 = mybir.dt.bfloat16
F8 = mybir.dt.float8e4
DR = mybir.MatmulPerfMode.DoubleRow

NT256 = L // 256    # 8 token chunks per batch for projections
NCHUNK = B * NT256  # 16 chunks in the linear stream
NI = L // 512       # 4 i-tiles per attention instance
NJ = L // 128       # 16 j-blocks

# fp8 scales: x*SX, w*SW quantized; product rescaled at PSUM drain
SX = 32.0
SWQ = 1024.0        # wq additionally carries 1/sqrt(HD)
SWK = 64.0
SWV = 64.0
QS = 1.0 / (SX * SWQ)
KS = 1.0 / (SX * SWK)
VS = 1.0 / (SX * SWV)


def build_nc():
    nc = bacc.Bacc(
        "TRN2", target_bir_lowering=False, debug=False, num_devices=NCORES
    )
    xq = nc.dram_tensor("xq", [NCHUNK, 128, 16, 512], F8, kind="ExternalInput").ap()
    wqh = nc.dram_tensor("wqh", [128, 16, DQ], F8, kind="ExternalInput").ap()
    wql = nc.dram_tensor("wql", [128, 16, DQ], F8, kind="ExternalInput").ap()
    wkh = nc.dram_tensor("wkh", [128, 16, DQ], F8, kind="ExternalInput").ap()
    wkl = nc.dram_tensor("wkl", [128, 16, DQ], F8, kind="ExternalInput").ap()
    wvh = nc.dram_tensor("wvh", [128, 16, DQ], F8, kind="ExternalInput").ap()
    wvl = nc.dram_tensor("wvl", [128, 16, DQ], F8, kind="ExternalInput").ap()
    wo = nc.dram_tensor("wo", [128, HPC, D], BF16, kind="ExternalInput").ap()
    # cc rows = [cos;cos], ss rows = [-sin;+sin] (bf16), for pairs (i, i+32)
    cc = nc.dram_tensor("cc", [ROPE, L], BF16, kind="ExternalInput").ap()
    ss = nc.dram_tensor("ss", [ROPE, L], BF16, kind="ExternalInput").ap()
    m4 = nc.dram_tensor("m4", [128, 4, 512], BF16, kind="ExternalInput").ap()
    y = nc.dram_tensor("y", [B, L, D], BF16, kind="ExternalOutput").ap()

    with tile.TileContext(nc) as tc:
        with (
            tc.tile_pool(name="consts", bufs=1) as consts,
            tc.tile_pool(name="wpool", bufs=1) as wpool,
            tc.tile_pool(name="qkv", bufs=1) as qkv,
            tc.tile_pool(name="xc", bufs=3) as xcpool,
            tc.tile_pool(name="et", bufs=3) as etpool,
            tc.tile_pool(name="rope", bufs=2) as ropepool,
            tc.tile_pool(name="ysb", bufs=2) as ypool,
            tc.tile_pool(name="riv", bufs=2) as rivpool,
            tc.tile_pool(name="rr", bufs=2) as rrpool,
            tc.tile_pool(name="rbc", bufs=1) as rbcpool,
            tc.tile_pool(name="pA", bufs=3, space="PSUM") as pA,
            tc.tile_pool(name="pST", bufs=2, space="PSUM") as pST,
            tc.tile_pool(name="pAV", bufs=2, space="PSUM") as pAV,
            tc.tile_pool(name="pR", bufs=1, space="PSUM") as pR,
        ):
            # ---- constants (no DMA deps; built first so PE warm-up can run) ----
            ones = consts.tile([128, 1], BF16)
            nc.vector.memset(ones, 1.0)
            dummy = consts.tile([128, 512], BF16)
            nc.vector.memset(dummy, 0.0)
            ident_f = consts.tile([128, 128], F32)
            masks.make_identity(nc, ident_f)
            ident = consts.tile([128, 128], F32R)
            nc.vector.tensor_copy(ident, ident_f)

            # PE warm-up: ~8 cheap matmuls spanning the ~3us p-state ramp
            pwarm = pA.tile([128, 512], F32, tag="pA", name="pwarm")
            for _ in range(8):
                nc.tensor.matmul(
                    pwarm[0:1, :], (ones), (dummy), start=True, stop=True
                )

            # ---- weights + tables (order sets DMA_ENGINES priority) ----
            wq_h = wpool.tile([128, 16, DQ], F8)
            wq_l = wpool.tile([128, 16, DQ], F8)
            wk_h = wpool.tile([128, 16, DQ], F8)
            wk_l = wpool.tile([128, 16, DQ], F8)
            wv_h = wpool.tile([128, 16, DQ], F8)
            wv_l = wpool.tile([128, 16, DQ], F8)
            wo_sb = wpool.tile([128, HPC, D], BF16)

            qt_sb = qkv.tile([128, HPC, L], BF16)   # [d, h, tok]
            kt_sb = qkv.tile([128, HPC, L], BF16)
            v_sb = qkv.tile([128, NJ, DQ], BF16)    # [tok_in_blk, jblk, hd]
            att_sb = qkv.tile([128, HPC, L], BF16)  # [hd, h, tok] NORMALIZED

            cc_sb = consts.tile([ROPE, L], BF16)
            ss_sb = consts.tile([ROPE, L], BF16)
            m4_sb = consts.tile([128, 4, 512], BF16)

            # ---- x chunk stream ----
            xc_tiles = {}

            def emit_xc_dma(i):
                if i >= NCHUNK or i in xc_tiles:
                    return
                t_ = xcpool.tile([128, 16, 512], F8, tag="xc", name=f"xc{i}")
                xc_tiles[i] = t_
                for hf in range(2):
                    nc.sync.dma_start(
                        out=t_[:, 8 * hf : 8 * hf + 8, :],
                        in_=xq[i, :, 8 * hf : 8 * hf + 8, :],
                    )

            # priming: interleave weight DMAs with the first x chunks so the
            # first projection chains unblock as early as possible
            nc.sync.dma_start(out=wq_h, in_=wqh)
            emit_xc_dma(0)
            nc.sync.dma_start(out=wq_l, in_=wql)
            nc.sync.dma_start(out=wk_h, in_=wkh)
            emit_xc_dma(1)
            nc.sync.dma_start(out=wk_l, in_=wkl)
            nc.sync.dma_start(out=wv_h, in_=wvh)
            nc.sync.dma_start(out=wv_l, in_=wvl)
            nc.sync.dma_start(out=cc_sb, in_=cc)
            nc.sync.dma_start(out=ss_sb, in_=ss)
            nc.sync.dma_start(out=m4_sb, in_=m4)

            def unit_wo():
                nc.sync.dma_start(out=wo_sb, in_=wo)

            # ---- y emission ----
            pending = []   # (b, t2, dd)
            ycnt = [0]
            ysb_tiles = {}

            def emit_y(b, t2, dd):
                p = pA.tile([128, 512], F32, tag="pA", name=f"yp_{b}_{t2}_{dd}")
                nc.tensor.matmul(
                    p,
                    (att_sb[:, 0, 128 * t2 : 128 * t2 + 128]),
                    (wo_sb[:, 0, 512 * dd : 512 * dd + 512]),
                    start=True,
                    stop=False,
                )
                nc.tensor.matmul(
                    p,
                    (att_sb[:, 1, 128 * t2 : 128 * t2 + 128]),
                    (wo_sb[:, 1, 512 * dd : 512 * dd + 512]),
                    start=False,
                    stop=True,
                )
                key = (b, t2)
                if key not in ysb_tiles:
                    ysb_tiles[key] = ypool.tile(
                        [128, 2048], BF16, tag="ysb", name=f"y_{b}_{t2}"
                    )
                yt = ysb_tiles[key]
                dst = yt[:, 512 * dd : 512 * dd + 512]
                if ycnt[0] % 2 == 0:
                    nc.scalar.activation(
                        dst, p, mybir.ActivationFunctionType.Copy
                    )
                else:
                    nc.vector.tensor_copy(dst, p)
                ycnt[0] += 1
                if dd == 3:
                    nc.gpsimd.dma_start(
                        out=y[b, 128 * t2 : 128 * t2 + 128, :], in_=yt
                    )
                    del ysb_tiles[key]

            def pop_y():
                if pending:
                    emit_y(*pending.pop(0))

            # ---- projection units (fp8 DoubleRow, 3-term hi/lo residual) ----
            # term order: (hi x, hi w), (lo x, hi w), (hi x, lo w) so the
            # chain can start before the lo-weight DMA lands
            def unit_qk_chain(i, w_hi, w_lo, out_sb, rt, drain_act, scale):
                def run():
                    xc = xc_tiles[i]
                    tt = i % NT256
                    pp = pA.tile([128, 512], F32, tag="pA")
                    pj = pp[:, 0:256]
                    n = 0
                    for w_, xo in ((w_hi, 0), (w_hi, 256), (w_lo, 0)):
                        for c in range(8):
                            nc.tensor.matmul(
                                pj,
                                (w_[:, 2 * c : 2 * c + 2,
                                    128 * rt : 128 * rt + 128]),
                                (xc[:, 2 * c : 2 * c + 2, xo : xo + 256]),
                                start=(n == 0),
                                stop=(n == 23),
                                perf_mode=DR,
                            )
                            n += 1
                    dst = out_sb[:, rt, 256 * tt : 256 * tt + 256]
                    if drain_act:
                        nc.scalar.activation(
                            dst, pj, mybir.ActivationFunctionType.Copy,
                            scale=scale,
                        )
                    else:
                        nc.vector.tensor_scalar_mul(dst, pj, scale)
                return run

            def unit_v_chain(i, ts2):
                def run():
                    xc = xc_tiles[i]
                    tt = i % NT256
                    pv = pA.tile([128, 512], F32, tag="pA")
                    pvj = pv[:, 0:256]
                    n = 0
                    for w_, xo in ((wv_h, 0), (wv_h, 256), (wv_l, 0)):
                        for c in range(8):
                            nc.tensor.matmul(
                                pvj,
                                (xc[:, 2 * c : 2 * c + 2,
                                    xo + 128 * ts2 : xo + 128 * ts2 + 128]),
                                (w_[:, 2 * c : 2 * c + 2, :]),
                                start=(n == 0),
                                stop=(n == 23),
                                perf_mode=DR,
                            )
                            n += 1
                    nc.vector.tensor_scalar_mul(v_sb[:, 2 * tt + ts2, :], pvj, VS)
                return run

            def unit_rope(out_sb, rt, qq, dq):
                # rot = [x1;x2]*[c;c] + [x2;x1]*[-s;s] on the 512-tok quarter
                # swap DMAs ride the ACT/DVE DGE queues so they never block
                # the x-chunk prefetches on the sync queue
                def run():
                    qsl = slice(512 * qq, 512 * qq + 512)
                    rope_rows = out_sb[0:ROPE, rt, qsl]
                    swap = ropepool.tile([ROPE, 512], BF16, tag="rope")
                    dq.dma_start(out=swap[0:32], in_=out_sb[32:64, rt, qsl])
                    dq.dma_start(out=swap[32:64], in_=out_sb[0:32, rt, qsl])
                    nc.vector.tensor_mul(swap, swap, ss_sb[:, qsl])
                    nc.vector.tensor_mul(rope_rows, rope_rows, cc_sb[:, qsl])
                    nc.vector.tensor_add(rope_rows, rope_rows, swap)
                return run

            def chunk_units(i):
                units = [lambda: emit_xc_dma(i + 2)]
                units.append(unit_qk_chain(i, wq_h, wq_l, qt_sb, 0, True, QS))
                units.append(unit_qk_chain(i, wq_h, wq_l, qt_sb, 1, True, QS))
                units.append(unit_qk_chain(i, wk_h, wk_l, kt_sb, 0, False, KS))
                units.append(unit_qk_chain(i, wk_h, wk_l, kt_sb, 1, False, KS))
                units.append(unit_v_chain(i, 0))
                units.append(unit_v_chain(i, 1))
                return units

            def chunk_qk_units(i):
                units = [lambda: emit_xc_dma(i + 2)]
                units.append(unit_qk_chain(i, wq_h, wq_l, qt_sb, 0, True, QS))
                units.append(unit_qk_chain(i, wq_h, wq_l, qt_sb, 1, True, QS))
                units.append(unit_qk_chain(i, wk_h, wk_l, kt_sb, 0, False, KS))
                units.append(unit_qk_chain(i, wk_h, wk_l, kt_sb, 1, False, KS))
                return units

            def chunk_v_units(i):
                return [unit_v_chain(i, 0), unit_v_chain(i, 1)]

            def rope_units(qq):
                out = []
                for rt in range(HPC):
                    out.append(unit_rope(qt_sb, rt, qq, nc.scalar))
                    out.append(unit_rope(kt_sb, rt, qq, nc.scalar))
                return out

            # ---- attention ----
            def att_instance(b, h, t, seg, last=False):
                av = pAV.tile([128, 512], F32, tag="pAV")
                # prt[:, 0:4] accumulates column sums (transposed denominators);
                # prt[0:1, :] is reused afterwards for the [1, 512] transpose
                prt = pR.tile([128, 512], F32, tag="pR")
                njb = 4 * t + 4
                for j in range(njb):
                    q = j - 4 * t
                    # exact causal widths: 512 / 384 / 256 / 128
                    off = 128 * q if q > 0 else 0
                    st = pST.tile([128, 512], F32, tag="pST")
                    nc.tensor.matmul(
                        st[:, off:512],
                        (kt_sb[:, h, 128 * j : 128 * j + 128]),
                        (qt_sb[:, h, 512 * t + off : 512 * t + 512]),
                        start=True,
                        stop=True,
                    )
                    et = etpool.tile([128, 512], BF16, tag="et")
                    nc.scalar.activation(
                        et[:, off:512],
                        st[:, off:512],
                        mybir.ActivationFunctionType.Exp,
                    )
                    if q >= 0:
                        # only cols [128q, 128q+128) are partially masked
                        # (beyond them causality always holds, mask==1)
                        msl = slice(128 * q, 128 * q + 128)
                        nc.vector.tensor_mul(
                            et[:, msl], et[:, msl], m4_sb[:, q, msl],
                        )
                    # fillers BEFORE rT/av: the ~4-deep OOO window parks on
                    # the exp->mask chain, so independent work must sit ahead
                    # of the parked dependents in program order
                    if seg:
                        seg.pop(0)()
                    pop_y()
                    # denominator column sums: et as stationary, out free = 1.
                    # PSUM pending-zero is 2KB-region granular, so only the
                    # very first matmul of the instance may carry start=True;
                    # later first-writes to other columns hit still-pending
                    # bytes and are zeroed-then-written by the hardware.
                    for ic in range(max(q, 0), 4):
                        nc.tensor.matmul(
                            prt[:, ic : ic + 1],
                            (et[:, 128 * ic : 128 * ic + 128]),
                            (ones),
                            start=(j == 0 and ic == 0),
                            stop=(j == 4 * t + ic),
                            skip_group_check=True,
                        )
                    nc.tensor.matmul(
                        av[:, off:512],
                        (v_sb[:, j, HD * h : HD * h + HD]),
                        (et[:, off:512]),
                        start=(j == 0),
                        stop=(j == njb - 1),
                        skip_group_check=True,
                    )
                # denominators: [128, 4] -> SBUF -> 4 PE transposes onto
                # partition 0 -> [1, 512] -> reciprocal -> partition broadcast
                rts = rivpool.tile([128, 4], F32R, tag="riv")
                nc.vector.tensor_copy(rts, prt[:, 0:4])
                prt_r = prt.bitcast(F32R)
                for ic in range(4):
                    # start=True only on the first transpose: one pending-zero
                    # mark for partition 0's row, later ones zero-then-write
                    nc.tensor.matmul(
                        prt_r[0:1, 128 * ic : 128 * ic + 128],
                        rts[:, ic : ic + 1],
                        ident,
                        is_transpose=True,
                        start=(ic == 0),
                        stop=(ic == 3),
                        skip_group_check=True,
                    )
                riv = rrpool.tile([1, 512], F32, tag="rr")
                nc.vector.reciprocal(riv, prt[0:1, 0:512])
                rbc = rbcpool.tile([128, 512], F32, tag="rbc")
                nc.gpsimd.partition_broadcast(rbc, riv)
                if last:
                    # split the normalize so the tail's first y tiles (which
                    # read 128-col slices of att) unblock after half the work
                    for hf in range(4):
                        sl = slice(128 * hf, 128 * hf + 128)
                        nc.vector.tensor_mul(
                            att_sb[:, h, 512 * t + 128 * hf :
                                   512 * t + 128 * hf + 128],
                            av[:, sl], rbc[:, sl],
                        )
                        if hf > 0:
                            pop_y()
                else:
                    nc.vector.tensor_mul(
                        att_sb[:, h, 512 * t : 512 * t + 512], av, rbc
                    )

            # ---------------- main schedule ----------------
            for b in range(B):
                base = NT256 * b
                emit_xc_dma(base)
                emit_xc_dma(base + 1)
                pro = chunk_units(base) + chunk_units(base + 1) + rope_units(0)
                if b == 0:
                    pro.append(unit_wo)
                for u in pro:
                    u()
                    pop_y()
                for t in range(NI):
                    if t < NI - 1:
                        seg = (
                            chunk_qk_units(base + 2 * t + 2)
                            + chunk_qk_units(base + 2 * t + 3)
                            + rope_units(t + 1)
                            + chunk_v_units(base + 2 * t + 2)
                            + chunk_v_units(base + 2 * t + 3)
                        )
                    else:
                        seg = []
                    for h in range(HPC):
                        att_instance(
                            b, h, t, seg,
                            last=(b == B - 1 and t == NI - 1 and h == HPC - 1),
                        )
                    while seg:
                        seg.pop(0)()
                        pop_y()
                    pending.extend(
                        (b, t2, dd)
                        for t2 in range(4 * t, 4 * t + 4)
                        for dd in range(4)
                    )
            while pending:
                emit_y(*pending.pop(0))
    nc.compile()
    return nc


_NC = None


def _get_nc():
    global _NC
    if _NC is None:
        _NC = build_nc()
    return _NC


def _fp8_split(a, s):
    import ml_dtypes

    f8 = ml_dtypes.float8_e4m3fn
    hi = (a * np.float32(s)).astype(f8)
    lo = ((a * np.float32(s)) - hi.astype(np.float32)).astype(f8)
    return hi, lo


def _host_inputs(x, mask, wq, wk, wv, wo):
    import ml_dtypes

    x = np.asarray(x, np.float32)
    wq = np.asarray(wq, np.float32)
    wk = np.asarray(wk, np.float32)
    wv = np.asarray(wv, np.float32)
    wo = np.asarray(wo, np.float32)

    # x chunks: [chunk, p, c, hi(256)|lo(256)] fp8, d = 128c + p
    xr = (
        x.transpose(0, 2, 1)          # [B, D, L]
        .reshape(B, 16, 128, NT256, 256)
        .transpose(0, 3, 2, 1, 4)     # [b, tt, p, c, tok]
    )
    xh, xl = _fp8_split(xr, SX)
    xq = np.ascontiguousarray(
        np.concatenate([xh, xl], axis=-1).reshape(NCHUNK, 128, 16, 512)
    )

    # permute head dims so RoPE pairs are (i, i+32): [evens, odds, pass]
    perm128 = np.concatenate(
        [np.arange(0, ROPE, 2), np.arange(1, ROPE, 2), np.arange(ROPE, HD)]
    )
    permD = np.concatenate([h * HD + perm128 for h in range(H)])
    wq_p = (wq * np.float32(1.0 / np.sqrt(HD)))[:, permD]
    wk_p = wk[:, permD]

    def wtile(w2d):  # [D, DQ] -> [128 p, 16 c, DQ]
        return np.ascontiguousarray(
            w2d.reshape(16, 128, DQ).transpose(1, 0, 2)
        )

    # RoPE tables, matching reference fp32 math (dim=64, repeat-2 interleave)
    ts_ = np.arange(0, ROPE, 2, dtype=np.float32)
    inv = (np.float32(10000.0) ** (-ts_ / np.float32(ROPE))).astype(np.float32)
    grid = np.arange(L, dtype=np.float32)[:, None] * inv[None, :]  # [L, 32]
    ccm = np.empty((ROPE, L), np.float32)
    ccm[0:32] = ccm[32:64] = np.cos(grid).T
    ssm = np.empty((ROPE, L), np.float32)
    ssm[0:32] = -np.sin(grid).T
    ssm[32:64] = np.sin(grid).T
    ccm = ccm.astype(ml_dtypes.bfloat16)
    ssm = ssm.astype(ml_dtypes.bfloat16)

    # diagonal-quarter masks for ST tiles: m4[j, q, i] = mask[i, 128q + j]
    mm = np.asarray(mask[0, 0, :512, :512])
    m4 = (
        mm.T.reshape(4, 128, 512).transpose(1, 0, 2).astype(ml_dtypes.bfloat16)
    )  # [j, q, i]

    in_maps = []
    for c in range(NCORES):
        sl = slice(DQ * c, DQ * c + DQ)
        wqh_, wql_ = _fp8_split(wtile(wq_p[:, sl]), SWQ)
        wkh_, wkl_ = _fp8_split(wtile(wk_p[:, sl]), SWK)
        wvh_, wvl_ = _fp8_split(wtile(wv[:, sl]), SWV)
        wo_t = np.ascontiguousarray(
            wo[sl, :].reshape(HPC, HD, D).transpose(1, 0, 2)
        ).astype(ml_dtypes.bfloat16)
        in_maps.append(
            {
                "xq": xq,
                "wqh": wqh_, "wql": wql_,
                "wkh": wkh_, "wkl": wkl_,
                "wvh": wvh_, "wvl": wvl_,
                "wo": wo_t,
                "cc": ccm,
                "ss": ssm,
                "m4": m4,
            }
        )
    return in_maps


def _reference_host(x, mask, wq, wk, wv, wo):
    """Exact-math numpy fallback (used only if the mask is not causal-tril)."""
    Hh, P = H, 64
    xx = np.asarray(x, np.float32)
    Bb, Ll, Dd = xx.shape
    K = Dd // Hh

    def rope(t):  # [b,h,s,d]
        d, s = t.shape[-1], t.shape[-2]
        ts_ = np.arange(0, d, 2, dtype=np.float32)
        inv = (np.float32(10000.0) ** (-ts_ / np.float32(d)))
        grid = np.arange(s, dtype=np.float32)[:, None] * inv[None, :]
        sin = np.repeat(np.sin(grid), 2, axis=-1)[None, None]
        cos = np.repeat(np.cos(grid), 2, axis=-1)[None, None]
        x1, x2 = t[..., ::2], t[..., 1::2]
        xs = np.stack([-x2, x1], axis=-1).reshape(t.shape)
        return t * cos + xs * sin

    def split(t):
        return t.reshape(Bb, Ll, Hh, K).transpose(0, 2, 1, 3)

    q = split(xx @ np.asarray(wq, np.float32)) / np.sqrt(K)
    q = np.concatenate([rope(q[..., :P]), q[..., P:]], axis=-1)
    k = split(xx @ np.asarray(wk, np.float32))
    k = np.concatenate([rope(k[..., :P]), k[..., P:]], axis=-1)
    v = split(xx @ np.asarray(wv, np.float32))
    s = np.einsum("bhik,bhjk->bhij", q, k)
    s = np.where(np.asarray(mask), s, np.float32(-1e8))
    s -= s.max(axis=-1, keepdims=True)
    e = np.exp(s)
    a = e / e.sum(axis=-1, keepdims=True)
    yy = np.einsum("bhij,bhjv->bhiv", a, v)
    yy = yy.transpose(0, 2, 1, 3).reshape(Bb, Ll, Dd)
    return (yy @ np.asarray(wo, np.float32)).astype(np.float32)


def kernel(**inputs):
    mask_arr = np.asarray(inputs["mask"])
    if not bool(
        (mask_arr[0, 0] == np.tril(np.ones((L, L), bool))).all()
    ):
        return _reference_host(
            inputs["x"], inputs["mask"], inputs["wq"], inputs["wk"],
            inputs["wv"], inputs["wo"],
        )
    nc = _get_nc()
    in_maps = _host_inputs(
        inputs["x"], inputs["mask"], inputs["wq"], inputs["wk"],
        inputs["wv"], inputs["wo"],
    )
    res = run_bass_kernel_spmd(nc, in_maps, core_ids=list(range(NCORES)))
    out = res.results[0]["y"].astype(np.float64)
    for c in range(1, NCORES):
        out += res.results[c]["y"]
    return out.astype(np.float32)


# revision 78
# speedup vs baseline: 1.2790x; 1.0115x over previous
"""Multi-head attention (B=2, L=2048, D=2048, H=16, causal + RoPE) on 8 TRN2 cores.

Sharding: tensor-parallel over heads. Core c owns heads {2c, 2c+1}:
  - wq/wk/wv column slices [D, 256], wo row slice [256, D]
  - each core computes a partial output y_c = att_c @ wo_c  (full shape)
  - host reduces: y = sum_c y_c   (the "all-reduce" of the output projection)

v3 schedule (vs v2 baseline):
  - QKV projections in fp8(e4m3) DoubleRow with a hi/lo residual split:
    x = (xh + xl)/SX, w = (wh + wl)/SW; x@w ~ (xh@wh + xl@wh + xh@wl)
    / (SX*SW).  3 DoubleRow chains of 8 MMs replace 16 fp32r MMs per
    256-token chain (24*128 vs 16*256 PE cycles).
  - Softmax denominators via et-as-STATIONARY matmuls: out [128i, 1]
    per 128-col slice of et (free-dim cost 1 instead of width), then a
    PE transpose of the accumulated [128,4] into [1,512], reciprocal,
    partition-broadcast, and the usual normalize-on-drain.
  - qt/kt/v/et/att all bf16: halves DVE element cost where 2x modes
    apply and lifts the fp32r >=256 moving-width restriction, so
    diagonal score blocks shrink to exact causal widths 512/384/256/128.
  - x chunks DMA'd as fp8 hi|lo packed [128,16,512] tiles (2 DMAs per
    chunk instead of 8, half the bytes of f32).
  - y written back per 128-token block [128, 2048] (1 DMA instead of 4).
  - PE warm-up: a few dummy matmuls at t~0 so the p-state ramp finishes
    before the first real projection chain.
"""

import glob
import os


def _ensure_env():
    # walrus_driver (neuronx-cc) must be on PATH for client-side NEFF compile.
    if not any("-b16-bazel-" in p for p in os.environ.get("PATH", "").split(":")):
        cands = sorted(glob.glob("/nix/store/*-b16-bazel-*/bin"))
        for c in cands:
            if os.path.exists(os.path.join(c, "neuronx-cc")) or glob.glob(
                os.path.join(c, "*walrus*")
            ):
                os.environ["PATH"] = c + ":" + os.environ["PATH"]
                break
        else:
            if cands:
                os.environ["PATH"] = cands[-1] + ":" + os.environ["PATH"]


_ensure_env()
os.environ.setdefault("JAX_COMPILATION_CACHE_DIR", "/tmp/jax_comp_cache")
os.environ.setdefault("JAX_PERSISTENT_CACHE_MIN_COMPILE_TIME_SECS", "1")
os.environ.setdefault("JAX_PERSISTENT_CACHE_MIN_ENTRY_SIZE_BYTES", "0")

import numpy as np  # noqa: E402

import concourse.bass as bass  # noqa: E402
import concourse.mybir as mybir  # noqa: E402
import concourse.tile as tile  # noqa: E402
from concourse import bacc, masks  # noqa: E402
from concourse.bass_utils import run_bass_kernel_spmd  # noqa: E402

NCORES = 8
B, L, D = 2, 2048, 2048
H = 16
HD = 128            # head dim
HPC = H // NCORES   # heads per core
DQ = HPC * HD       # 256: per-core projection width
ROPE = 64           # RoPE dims per head
F32 = mybir.dt.float32
F32R = mybir.dt.float32r
BF16 = mybir.dt.bfloat16
F8 = mybir.dt.float8e4
DR = mybir.MatmulPerfMode.DoubleRow

NT256 = L // 256    # 8 token chunks per batch for projections
NCHUNK = B * NT256  # 16 chunks in the linear stream
NI = L // 512       # 4 i-tiles per attention instance
NJ = L // 128       # 16 j-blocks

# fp8 scales: x*SX, w*SW quantized; product rescaled at PSUM drain
SX = 32.0
SWQ = 1024.0        # wq additionally carries 1/sqrt(HD)
SWK = 64.0
SWV = 64.0
QS = 1.0 / (SX * SWQ)
KS = 1.0 / (SX * SWK)
VS = 1.0 / (SX * SWV)
# att_sb holds SA*att (the softmax-denominator reciprocal is pre-scaled by
# SA via the ones constant) so its fp8 hi/lo split sits in normal range;
# wo is quantized at SWO. y-emit PSUM drains rescale by 1/(SA*SWO).
SA = 32.0
SWO = 64.0
YS8 = 1.0 / (SA * SWO)   # fp8 DoubleRow y-emit drain scale
YSB = 1.0 / SA           # bf16 (last tile) y-emit drain scale


def build_nc():
    nc = bacc.Bacc(
        "TRN2", target_bir_lowering=False, debug=False, num_devices=NCORES
    )
    xq = nc.dram_tensor("xq", [NCHUNK, 128, 16, 512], F8, kind="ExternalInput").ap()
    # weights packed hi|lo along the last axis: [:, :, 0:DQ]=hi, [DQ:2DQ]=lo
    wq8 = nc.dram_tensor("wq8", [128, 16, 2 * DQ], F8, kind="ExternalInput").ap()
    wk8 = nc.dram_tensor("wk8", [128, 16, 2 * DQ], F8, kind="ExternalInput").ap()
    wv8 = nc.dram_tensor("wv8", [128, 16, 2 * DQ], F8, kind="ExternalInput").ap()
    wo = nc.dram_tensor("wo", [128, HPC, D], BF16, kind="ExternalInput").ap()
    # cc rows = [cos;cos], ss rows = [-sin;+sin] (bf16), for pairs (i, i+32)
    cc = nc.dram_tensor("cc", [ROPE, L], BF16, kind="ExternalInput").ap()
    ss = nc.dram_tensor("ss", [ROPE, L], BF16, kind="ExternalInput").ap()
    m4 = nc.dram_tensor("m4", [128, 4, 512], BF16, kind="ExternalInput").ap()
    y = nc.dram_tensor("y", [B, L, D], BF16, kind="ExternalOutput").ap()

    with tile.TileContext(nc) as tc:
        with (
            tc.tile_pool(name="consts", bufs=1) as consts,
            tc.tile_pool(name="wpool", bufs=1) as wpool,
            tc.tile_pool(name="qkv", bufs=1) as qkv,
            tc.tile_pool(name="xc", bufs=4) as xcpool,
            tc.tile_pool(name="et", bufs=3) as etpool,
            tc.tile_pool(name="rope", bufs=4) as ropepool,
            tc.tile_pool(name="ysb", bufs=2) as ypool,
            tc.tile_pool(name="riv", bufs=2) as rivpool,
            tc.tile_pool(name="rr", bufs=2) as rrpool,
            tc.tile_pool(name="rbc", bufs=1) as rbcpool,
            tc.tile_pool(name="pA", bufs=3, space="PSUM") as pA,
            tc.tile_pool(name="pST", bufs=2, space="PSUM") as pST,
            tc.tile_pool(name="pAV", bufs=2, space="PSUM") as pAV,
            tc.tile_pool(name="pR", bufs=1, space="PSUM") as pR,
        ):
            # ---- constants (no DMA deps; built first so PE warm-up can run) ----
            # ones carries 1/SA so the denominator sums come out pre-scaled:
            # riv = SA/r and the normalized att tiles hold SA*att (fp8 range)
            ones = consts.tile([128, 1], BF16)
            nc.vector.memset(ones, 1.0 / SA)
            dummy = consts.tile([128, 512], BF16)
            nc.vector.memset(dummy, 0.0)
            ident_f = consts.tile([128, 128], F32)
            masks.make_identity(nc, ident_f)
            ident = consts.tile([128, 128], F32R)
            nc.vector.tensor_copy(ident, ident_f)

            # PE warm-up: cheap matmuls spanning the ~3us p-state ramp plus
            # the DMA-supply-bound stretch before the first chains unblock
            pwarm = pA.tile([128, 512], F32, tag="pA", name="pwarm")
            for _ in range(12):
                nc.tensor.matmul(
                    pwarm[0:1, :], (ones), (dummy), start=True, stop=True
                )

            # ---- weights + tables (order sets DMA_ENGINES priority) ----
            wq_sb = wpool.tile([128, 16, 2 * DQ], F8)
            wk_sb = wpool.tile([128, 16, 2 * DQ], F8)
            wv_sb = wpool.tile([128, 16, 2 * DQ], F8)
            wo_sb = wpool.tile([128, HPC, D], BF16)

            qt_sb = qkv.tile([128, HPC, L], BF16)   # [d, h, tok]
            kt_sb = qkv.tile([128, HPC, L], BF16)
            v_sb = qkv.tile([128, NJ, DQ], BF16)    # [tok_in_blk, jblk, hd]
            att_sb = qkv.tile([128, HPC, L], BF16)  # [hd, h, tok] = SA*att

            cc_sb = consts.tile([ROPE, L], BF16)
            ss_sb = consts.tile([ROPE, L], BF16)
            m4_sb = consts.tile([128, 4, 512], BF16)

            # ---- x chunk stream ----
            xc_tiles = {}

            def emit_xc_dma(i, half=None):
                # half=None: emit both halves, but only once per chunk;
                # half=0/1: priming-time single-half emission
                if i >= NCHUNK or (i in xc_tiles and half is None):
                    return
                if i in xc_tiles:
                    t_ = xc_tiles[i]
                else:
                    t_ = xcpool.tile([128, 16, 512], F8, tag="xc", name=f"xc{i}")
                    xc_tiles[i] = t_
                halves = range(2) if half is None else [half]
                for hf in halves:
                    nc.sync.dma_start(
                        out=t_[:, 8 * hf : 8 * hf + 8, :],
                        in_=xq[i, :, 8 * hf : 8 * hf + 8, :],
                    )

            # priming: DMA order matches the PE consumption order of the
            # first two chunks' chains so supply granularity stalls stay small
            nc.sync.dma_start(out=wq_sb, in_=wq8)
            emit_xc_dma(0, 0)
            emit_xc_dma(0, 1)
            nc.sync.dma_start(out=wk_sb, in_=wk8)
            emit_xc_dma(1, 0)
            emit_xc_dma(1, 1)
            emit_xc_dma(2)
            emit_xc_dma(3)
            nc.sync.dma_start(out=wv_sb, in_=wv8)
            nc.sync.dma_start(out=cc_sb, in_=cc)
            nc.sync.dma_start(out=ss_sb, in_=ss)
            nc.sync.dma_start(out=m4_sb, in_=m4)
            nc.sync.dma_start(out=wo_sb, in_=wo)

            # ---- y emission ----
            pending = []   # (b, t2, dd) poppable now
            staged = []    # (b, t2, dd) from the current tile, not yet poppable
            ycnt = [0]
            ysb_tiles = {}

            def emit_y(b, t2, dd):
                p = pA.tile([128, 512], F32, tag="pA", name=f"yp_{b}_{t2}_{dd}")
                tsl = slice(128 * t2, 128 * t2 + 128)
                dsl = slice(512 * dd, 512 * dd + 512)
                tail = b == B - 1 and t2 >= 4 * (NI - 1)
                nc.tensor.matmul(
                    p, (att_sb[:, 0, tsl]), (wo_sb[:, 0, dsl]),
                    start=True, stop=False,
                )
                nc.tensor.matmul(
                    p, (att_sb[:, 1, tsl]), (wo_sb[:, 1, dsl]),
                    start=False, stop=True,
                )
                key = (b, t2)
                if key not in ysb_tiles:
                    ysb_tiles[key] = ypool.tile(
                        [128, 2048], BF16, tag="ysb", name=f"y_{b}_{t2}"
                    )
                yt = ysb_tiles[key]
                dst = yt[:, 512 * dd : 512 * dd + 512]
                ysc = YSB
                final_slice = tail and t2 == 4 * NI - 1 and dd == 3
                if final_slice:
                    # the very last drain goes wholly to ACT: no waiting on
                    # the (laggier) DVE before the final DMA can launch
                    nc.scalar.activation(
                        dst, p, mybir.ActivationFunctionType.Copy, scale=ysc
                    )
                elif tail:
                    # drains are the tail throughput limit: split each across
                    # ACT and DVE so they keep pace with the matmuls
                    nc.scalar.activation(
                        dst[:, 0:256], p[:, 0:256],
                        mybir.ActivationFunctionType.Copy, scale=ysc,
                    )
                    nc.vector.tensor_scalar_mul(
                        dst[:, 256:512], p[:, 256:512], ysc
                    )
                elif ycnt[0] % 2 == 0:
                    nc.scalar.activation(
                        dst, p, mybir.ActivationFunctionType.Copy, scale=ysc
                    )
                else:
                    nc.vector.tensor_scalar_mul(dst, p, ysc)
                ycnt[0] += 1
                if tail and t2 == 4 * NI - 1:
                    # very last block: stream each 512-slice immediately on
                    # alternating HWDGE queues, minimizing the final chain
                    dq = nc.scalar if dd % 2 == 0 else nc.sync
                    dq.dma_start(
                        out=y[b, 128 * t2 : 128 * t2 + 128,
                              512 * dd : 512 * dd + 512],
                        in_=dst,
                    )
                    if dd == 3:
                        del ysb_tiles[key]
                elif tail:
                    # last tile: stream halves out early on two queues so the
                    # final writeback isn't one serialized chain at the end
                    if dd == 1:
                        nc.sync.dma_start(
                            out=y[b, 128 * t2 : 128 * t2 + 128, 0:1024],
                            in_=yt[:, 0:1024],
                        )
                    elif dd == 3:
                        nc.gpsimd.dma_start(
                            out=y[b, 128 * t2 : 128 * t2 + 128, 1024:2048],
                            in_=yt[:, 1024:2048],
                        )
                        del ysb_tiles[key]
                elif dd == 3:
                    nc.gpsimd.dma_start(
                        out=y[b, 128 * t2 : 128 * t2 + 128, :], in_=yt
                    )
                    del ysb_tiles[key]

            def pop_y():
                if pending:
                    emit_y(*pending.pop(0))

            # ---- projection units (fp8 DoubleRow, 3-term hi/lo residual) ----
            # terms: (hi x, hi w), (lo x, hi w), (hi x, lo w)
            def unit_qk_chain(i, w_sb, out_sb, rt, drain_act, scale):
                def run():
                    xc = xc_tiles[i]
                    tt = i % NT256
                    pp = pA.tile([128, 512], F32, tag="pA")
                    pj = pp[:, 0:256]
                    n = 0
                    for wo_, xo in ((0, 0), (0, 256), (DQ, 0)):
                        for c in range(8):
                            nc.tensor.matmul(
                                pj,
                                (w_sb[:, 2 * c : 2 * c + 2,
                                      wo_ + 128 * rt : wo_ + 128 * rt + 128]),
                                (xc[:, 2 * c : 2 * c + 2, xo : xo + 256]),
                                start=(n == 0),
                                stop=(n == 23),
                                perf_mode=DR,
                            )
                            n += 1
                    dst = out_sb[:, rt, 256 * tt : 256 * tt + 256]
                    if drain_act:
                        nc.scalar.activation(
                            dst, pj, mybir.ActivationFunctionType.Copy,
                            scale=scale,
                        )
                    else:
                        nc.vector.tensor_scalar_mul(dst, pj, scale)
                return run

            def unit_v_chain(i, ts2):
                def run():
                    xc = xc_tiles[i]
                    tt = i % NT256
                    pv = pA.tile([128, 512], F32, tag="pA")
                    pvj = pv[:, 0:256]
                    n = 0
                    for wo_, xo in ((0, 0), (0, 256), (DQ, 0)):
                        for c in range(8):
                            nc.tensor.matmul(
                                pvj,
                                (xc[:, 2 * c : 2 * c + 2,
                                    xo + 128 * ts2 : xo + 128 * ts2 + 128]),
                                (wv_sb[:, 2 * c : 2 * c + 2,
                                       wo_ : wo_ + DQ]),
                                start=(n == 0),
                                stop=(n == 23),
                                perf_mode=DR,
                            )
                            n += 1
                    nc.vector.tensor_scalar_mul(v_sb[:, 2 * tt + ts2, :], pvj, VS)
                return run

            # rot = [x1;x2]*[c;c] + [x2;x1]*[-s;s] on the 512-tok quarter.
            # Swap DMAs and muls are SEPARATE units: the tiny swap DMAs must
            # enter the DMA-engine FIFO before the 1.4us x-chunk prefetch
            # transfers of the same seg, or the muls (and the next tile's
            # first STs) stall ~3-4us behind them.
            rope_swaps = {}

            def unit_rope_swap(out_sb, rt, qq, key, dq):
                def run():
                    qsl = slice(512 * qq, 512 * qq + 512)
                    swap = ropepool.tile(
                        [ROPE, 512], BF16, tag="rope", name=f"sw{key}_{qq}"
                    )
                    rope_swaps[key] = swap
                    dq.dma_start(out=swap[0:32], in_=out_sb[32:64, rt, qsl])
                    dq.dma_start(out=swap[32:64], in_=out_sb[0:32, rt, qsl])
                return run

            def unit_rope_mul(out_sb, rt, qq, key):
                def run():
                    qsl = slice(512 * qq, 512 * qq + 512)
                    rope_rows = out_sb[0:ROPE, rt, qsl]
                    swap = rope_swaps.pop(key)
                    nc.vector.tensor_mul(swap, swap, ss_sb[:, qsl])
                    nc.vector.tensor_mul(rope_rows, rope_rows, cc_sb[:, qsl])
                    nc.vector.tensor_add(rope_rows, rope_rows, swap)
                return run

            def _qk4(i):
                return [
                    unit_qk_chain(i, wq_sb, qt_sb, 0, True, QS),
                    unit_qk_chain(i, wq_sb, qt_sb, 1, True, QS),
                    unit_qk_chain(i, wk_sb, kt_sb, 0, False, KS),
                    unit_qk_chain(i, wk_sb, kt_sb, 1, False, KS),
                ]

            def prefetch_units(i):
                # issued after the rope units: the 2.9us chunk DMAs must not
                # get ahead of the tiny rope-swap DMAs on the shared engines,
                # but still land before the NEXT seg's chains need them
                return [lambda: emit_xc_dma(i), lambda: emit_xc_dma(i + 1)]

            def chunk_v_units(i):
                return [unit_v_chain(i, 0), unit_v_chain(i, 1)]

            def rope_swap_units(qq):
                out = []
                for rt in range(HPC):
                    out.append(
                        unit_rope_swap(qt_sb, rt, qq, f"q{rt}", nc.scalar)
                    )
                    out.append(
                        unit_rope_swap(kt_sb, rt, qq, f"k{rt}", nc.scalar)
                    )
                return out

            def rope_mul_units(qq):
                out = []
                for rt in range(HPC):
                    out.append(unit_rope_mul(qt_sb, rt, qq, f"q{rt}"))
                    out.append(unit_rope_mul(kt_sb, rt, qq, f"k{rt}"))
                return out

            def rope_units(qq):
                return rope_swap_units(qq) + rope_mul_units(qq)

            # ---- attention ----
            def att_instance(b, h, t, seg, last=False, hold=0):
                # hold: skip pop_y on that many of every 2 j-steps, saving
                # pending y-emits to cover this/next instance's drain window
                av = pAV.tile([128, 512], F32, tag="pAV")
                # prt[:, 0:4] accumulates column sums (transposed denominators);
                # prt[0:1, :] is reused afterwards for the [1, 512] transpose
                prt = pR.tile([128, 512], F32, tag="pR")
                njb = 4 * t + 4
                for j in range(njb):
                    q = j - 4 * t
                    # exact causal widths: 512 / 384 / 256 / 128
                    off = 128 * q if q > 0 else 0
                    st = pST.tile([128, 512], F32, tag="pST")
                    nc.tensor.matmul(
                        st[:, off:512],
                        (kt_sb[:, h, 128 * j : 128 * j + 128]),
                        (qt_sb[:, h, 512 * t + off : 512 * t + 512]),
                        start=True,
                        stop=True,
                    )
                    et = etpool.tile([128, 512], BF16, tag="et")
                    nc.scalar.activation(
                        et[:, off:512],
                        st[:, off:512],
                        mybir.ActivationFunctionType.Exp,
                    )
                    if q >= 0:
                        # only cols [128q, 128q+128) are partially masked
                        # (beyond them causality always holds, mask==1)
                        msl = slice(128 * q, 128 * q + 128)
                        nc.vector.tensor_mul(
                            et[:, msl], et[:, msl], m4_sb[:, q, msl],
                        )
                    # fillers BEFORE rT/av: the ~4-deep OOO window parks on
                    # the exp->mask chain, so independent work must sit ahead
                    # of the parked dependents in program order
                    if seg:
                        seg.pop(0)()
                    if hold == 0 or (hold == 1 and j % 2 == 0):
                        pop_y()
                    # denominator column sums: et as stationary, out free = 1.
                    # PSUM pending-zero is 2KB-region granular, so only the
                    # very first matmul of the instance may carry start=True;
                    # later first-writes to other columns hit still-pending
                    # bytes and are zeroed-then-written by the hardware.
                    for ic in range(max(q, 0), 4):
                        nc.tensor.matmul(
                            prt[:, ic : ic + 1],
                            (et[:, 128 * ic : 128 * ic + 128]),
                            (ones),
                            start=(j == 0 and ic == 0),
                            stop=(j == 4 * t + ic),
                            skip_group_check=True,
                        )
                    nc.tensor.matmul(
                        av[:, off:512],
                        (v_sb[:, j, HD * h : HD * h + HD]),
                        (et[:, off:512]),
                        start=(j == 0),
                        stop=(j == njb - 1),
                        skip_group_check=True,
                    )
                # denominators: [128, 4] -> SBUF -> 4 PE transposes onto
                # partition 0 -> [1, 512] -> reciprocal -> partition broadcast
                rts = rivpool.tile([128, 4], F32R, tag="riv")
                nc.vector.tensor_copy(rts, prt[:, 0:4])
                # ready y-emit matmuls BEFORE the transposes: the transposes
                # stall ~0.5us on the rts copy and PE executes in order
                pop_y()
                pop_y()
                prt_r = prt.bitcast(F32R)
                for ic in range(4):
                    # start=True only on the first transpose: one pending-zero
                    # mark for partition 0's row, later ones zero-then-write
                    nc.tensor.matmul(
                        prt_r[0:1, 128 * ic : 128 * ic + 128],
                        rts[:, ic : ic + 1],
                        ident,
                        is_transpose=True,
                        start=(ic == 0),
                        stop=(ic == 3),
                        skip_group_check=True,
                    )
                if hold:
                    # held-back emits land here, filling the PE while the
                    # reciprocal/broadcast/normalize chain runs
                    pop_y()
                    pop_y()
                    pop_y()
                    pop_y()
                rbc = rbcpool.tile([128, 512], F32, tag="rbc")
                if last:
                    # split reciprocal/broadcast/normalize per quarter so the
                    # tail's first y tiles unblock as early as possible
                    riv = rrpool.tile([1, 512], F32, tag="rr")
                    for hf in range(4):
                        sl = slice(128 * hf, 128 * hf + 128)
                        nc.vector.reciprocal(riv[:, sl], prt[0:1, sl])
                        nc.gpsimd.partition_broadcast(rbc[:, sl], riv[:, sl])
                        nc.vector.tensor_mul(
                            att_sb[:, h, 512 * t + 128 * hf :
                                   512 * t + 128 * hf + 128],
                            av[:, sl], rbc[:, sl],
                        )
                        if hf > 0:
                            pop_y()
                else:
                    riv = rrpool.tile([1, 512], F32, tag="rr")
                    nc.vector.reciprocal(riv, prt[0:1, 0:512])
                    nc.gpsimd.partition_broadcast(rbc, riv)
                    sl_t = slice(512 * t, 512 * t + 512)
                    nc.vector.tensor_mul(att_sb[:, h, sl_t], av, rbc)

            # ---------------- main schedule ----------------
            # The qk chains for quarter q run one attention-tile EARLIER than
            # quarter q's attention (chains for q=0,1 in the prologue, chains
            # for q+1 in seg of tile q-1... i.e. seg_t carries chains(t+2)),
            # so rope units for tile t+1 sit at the HEAD of seg_t with their
            # inputs already drained — their swap-DMA + mul latency hides
            # under a whole tile of attention instead of stalling tile t+1.
            for b in range(B):
                base = NT256 * b
                emit_xc_dma(base)
                emit_xc_dma(base + 1)
                pro = _qk4(base) + _qk4(base + 1) + _qk4(base + 2) + _qk4(
                    base + 3
                )
                pro_pop = (
                    rope_units(0)
                    + chunk_v_units(base)
                    + chunk_v_units(base + 1)
                    + prefetch_units(base + 4)
                )
                if b > 0:
                    pro_pop = prefetch_units(base + 2) + pro_pop
                for u in pro:
                    u()
                pending.extend(staged)
                staged.clear()
                for u in pro_pop:
                    u()
                    pop_y()
                    pop_y()
                for t in range(NI):
                    # emits staged by tile t-1 become poppable now: their
                    # att8 prep races only the first few j-steps (gated)
                    pending.extend(staged)
                    staged.clear()
                    if t < NI - 2:
                        seg = (
                            rope_units(t + 1)
                            + _qk4(base + 2 * t + 4)
                            + _qk4(base + 2 * t + 5)
                            + prefetch_units(base + 2 * t + 6)
                            + chunk_v_units(base + 2 * t + 2)
                            + chunk_v_units(base + 2 * t + 3)
                        )
                    elif t == NI - 2:
                        seg = (
                            rope_units(t + 1)
                            + prefetch_units(base + 2 * t + 6)
                            + chunk_v_units(base + 2 * t + 2)
                            + chunk_v_units(base + 2 * t + 3)
                        )
                    else:
                        seg = []
                    final_tile = b == B - 1 and t == NI - 1
                    for h in range(HPC):
                        att_instance(
                            b, h, t, seg,
                            last=(final_tile and h == HPC - 1),
                            hold=(h + 1 if final_tile else 0),
                        )
                    staged.extend(
                        (b, t2, dd)
                        for t2 in range(4 * t, 4 * t + 4)
                        for dd in range(4)
                    )
                    while seg:
                        seg.pop(0)()
            pending.extend(staged)
            staged.clear()
            while pending:
                emit_y(*pending.pop(0))
    nc.compile()
    return nc


_NC = None


def _get_nc():
    global _NC
    if _NC is None:
        _NC = build_nc()
    return _NC


def _fp8_split(a, s):
    import ml_dtypes

    f8 = ml_dtypes.float8_e4m3fn
    hi = (a * np.float32(s)).astype(f8)
    lo = ((a * np.float32(s)) - hi.astype(np.float32)).astype(f8)
    return hi, lo


def _host_inputs(x, mask, wq, wk, wv, wo):
    import ml_dtypes

    x = np.asarray(x, np.float32)
    wq = np.asarray(wq, np.float32)
    wk = np.asarray(wk, np.float32)
    wv = np.asarray(wv, np.float32)
    wo = np.asarray(wo, np.float32)

    # x chunks: [chunk, p, c, hi(256)|lo(256)] fp8, d = 128c + p
    xr = (
        x.transpose(0, 2, 1)          # [B, D, L]
        .reshape(B, 16, 128, NT256, 256)
        .transpose(0, 3, 2, 1, 4)     # [b, tt, p, c, tok]
    )
    xh, xl = _fp8_split(xr, SX)
    xq = np.ascontiguousarray(
        np.concatenate([xh, xl], axis=-1).reshape(NCHUNK, 128, 16, 512)
    )

    # permute head dims so RoPE pairs are (i, i+32): [evens, odds, pass]
    perm128 = np.concatenate(
        [np.arange(0, ROPE, 2), np.arange(1, ROPE, 2), np.arange(ROPE, HD)]
    )
    permD = np.concatenate([h * HD + perm128 for h in range(H)])
    wq_p = (wq * np.float32(1.0 / np.sqrt(HD)))[:, permD]
    wk_p = wk[:, permD]

    def wtile(w2d):  # [D, DQ] -> [128 p, 16 c, DQ]
        return np.ascontiguousarray(
            w2d.reshape(16, 128, DQ).transpose(1, 0, 2)
        )

    # RoPE tables, matching reference fp32 math (dim=64, repeat-2 interleave)
    ts_ = np.arange(0, ROPE, 2, dtype=np.float32)
    inv = (np.float32(10000.0) ** (-ts_ / np.float32(ROPE))).astype(np.float32)
    grid = np.arange(L, dtype=np.float32)[:, None] * inv[None, :]  # [L, 32]
    ccm = np.empty((ROPE, L), np.float32)
    ccm[0:32] = ccm[32:64] = np.cos(grid).T
    ssm = np.empty((ROPE, L), np.float32)
    ssm[0:32] = -np.sin(grid).T
    ssm[32:64] = np.sin(grid).T
    ccm = ccm.astype(ml_dtypes.bfloat16)
    ssm = ssm.astype(ml_dtypes.bfloat16)

    # diagonal-quarter masks for ST tiles: m4[j, q, i] = mask[i, 128q + j]
    mm = np.asarray(mask[0, 0, :512, :512])
    m4 = (
        mm.T.reshape(4, 128, 512).transpose(1, 0, 2).astype(ml_dtypes.bfloat16)
    )  # [j, q, i]

    in_maps = []
    for c in range(NCORES):
        sl = slice(DQ * c, DQ * c + DQ)
        def pack_hl(w2d, s):
            hi, lo = _fp8_split(wtile(w2d), s)
            return np.ascontiguousarray(np.concatenate([hi, lo], axis=-1))

        wq8_ = pack_hl(wq_p[:, sl], SWQ)
        wk8_ = pack_hl(wk_p[:, sl], SWK)
        wv8_ = pack_hl(wv[:, sl], SWV)
        wo_t = np.ascontiguousarray(
            wo[sl, :].reshape(HPC, HD, D).transpose(1, 0, 2)
        ).astype(ml_dtypes.bfloat16)
        in_maps.append(
            {
                "xq": xq,
                "wq8": wq8_, "wk8": wk8_, "wv8": wv8_,
                "wo": wo_t,
                "cc": ccm,
                "ss": ssm,
                "m4": m4,
            }
        )
    return in_maps


def _reference_host(x, mask, wq, wk, wv, wo):
    """Exact-math numpy fallback (used only if the mask is not causal-tril)."""
    Hh, P = H, 64
    xx = np.asarray(x, np.float32)
    Bb, Ll, Dd = xx.shape
    K = Dd // Hh

    def rope(t):  # [b,h,s,d]
        d, s = t.shape[-1], t.shape[-2]
        ts_ = np.arange(0, d, 2, dtype=np.float32)
        inv = (np.float32(10000.0) ** (-ts_ / np.float32(d)))
        grid = np.arange(s, dtype=np.float32)[:, None] * inv[None, :]
        sin = np.repeat(np.sin(grid), 2, axis=-1)[None, None]
        cos = np.repeat(np.cos(grid), 2, axis=-1)[None, None]
        x1, x2 = t[..., ::2], t[..., 1::2]
        xs = np.stack([-x2, x1], axis=-1).reshape(t.shape)
        return t * cos + xs * sin

    def split(t):
        return t.reshape(Bb, Ll, Hh, K).transpose(0, 2, 1, 3)

    q = split(xx @ np.asarray(wq, np.float32)) / np.sqrt(K)
    q = np.concatenate([rope(q[..., :P]), q[..., P:]], axis=-1)
    k = split(xx @ np.asarray(wk, np.float32))
    k = np.concatenate([rope(k[..., :P]), k[..., P:]], axis=-1)
    v = split(xx @ np.asarray(wv, np.float32))
    s = np.einsum("bhik,bhjk->bhij", q, k)
    s = np.where(np.asarray(mask), s, np.float32(-1e8))
    s -= s.max(axis=-1, keepdims=True)
    e = np.exp(s)
    a = e / e.sum(axis=-1, keepdims=True)
    yy = np.einsum("bhij,bhjv->bhiv", a, v)
    yy = yy.transpose(0, 2, 1, 3).reshape(Bb, Ll, Dd)
    return (yy @ np.asarray(wo, np.float32)).astype(np.float32)


def kernel(**inputs):
    mask_arr = np.asarray(inputs["mask"])
    if not bool(
        (mask_arr[0, 0] == np.tril(np.ones((L, L), bool))).all()
    ):
        return _reference_host(
            inputs["x"], inputs["mask"], inputs["wq"], inputs["wk"],
            inputs["wv"], inputs["wo"],
        )
    nc = _get_nc()
    in_maps = _host_inputs(
        inputs["x"], inputs["mask"], inputs["wq"], inputs["wk"],
        inputs["wv"], inputs["wo"],
    )
    res = run_bass_kernel_spmd(nc, in_maps, core_ids=list(range(NCORES)))
    out = res.results[0]["y"].astype(np.float64)
    for c in range(1, NCORES):
        out += res.results[c]["y"]
    return out.astype(np.float32)


# revision 82
# speedup vs baseline: 1.2839x; 1.0038x over previous
"""Multi-head attention (B=2, L=2048, D=2048, H=16, causal + RoPE) on 8 TRN2 cores.

Sharding: tensor-parallel over heads. Core c owns heads {2c, 2c+1}:
  - wq/wk/wv column slices [D, 256], wo row slice [256, D]
  - each core computes a partial output y_c = att_c @ wo_c  (full shape)
  - host reduces: y = sum_c y_c   (the "all-reduce" of the output projection)

v3 schedule (vs v2 baseline):
  - QKV projections in fp8(e4m3) DoubleRow with a hi/lo residual split:
    x = (xh + xl)/SX, w = (wh + wl)/SW; x@w ~ (xh@wh + xl@wh + xh@wl)
    / (SX*SW).  3 DoubleRow chains of 8 MMs replace 16 fp32r MMs per
    256-token chain (24*128 vs 16*256 PE cycles).
  - Softmax denominators via et-as-STATIONARY matmuls: out [128i, 1]
    per 128-col slice of et (free-dim cost 1 instead of width), then a
    PE transpose of the accumulated [128,4] into [1,512], reciprocal,
    partition-broadcast, and the usual normalize-on-drain.
  - qt/kt/v/et/att all bf16: halves DVE element cost where 2x modes
    apply and lifts the fp32r >=256 moving-width restriction, so
    diagonal score blocks shrink to exact causal widths 512/384/256/128.
  - x chunks DMA'd as fp8 hi|lo packed [128,16,512] tiles (2 DMAs per
    chunk instead of 8, half the bytes of f32).
  - y written back per 128-token block [128, 2048] (1 DMA instead of 4).
  - PE warm-up: a few dummy matmuls at t~0 so the p-state ramp finishes
    before the first real projection chain.
"""

import glob
import os


def _ensure_env():
    # walrus_driver (neuronx-cc) must be on PATH for client-side NEFF compile.
    if not any("-b16-bazel-" in p for p in os.environ.get("PATH", "").split(":")):
        cands = sorted(glob.glob("/nix/store/*-b16-bazel-*/bin"))
        for c in cands:
            if os.path.exists(os.path.join(c, "neuronx-cc")) or glob.glob(
                os.path.join(c, "*walrus*")
            ):
                os.environ["PATH"] = c + ":" + os.environ["PATH"]
                break
        else:
            if cands:
                os.environ["PATH"] = cands[-1] + ":" + os.environ["PATH"]


_ensure_env()
os.environ.setdefault("JAX_COMPILATION_CACHE_DIR", "/tmp/jax_comp_cache")
os.environ.setdefault("JAX_PERSISTENT_CACHE_MIN_COMPILE_TIME_SECS", "1")
os.environ.setdefault("JAX_PERSISTENT_CACHE_MIN_ENTRY_SIZE_BYTES", "0")

import numpy as np  # noqa: E402

import concourse.bass as bass  # noqa: E402
import concourse.mybir as mybir  # noqa: E402
import concourse.tile as tile  # noqa: E402
from concourse import bacc, masks  # noqa: E402
from concourse.bass_utils import run_bass_kernel_spmd  # noqa: E402

NCORES = 8
B, L, D = 2, 2048, 2048
H = 16
HD = 128            # head dim
HPC = H // NCORES   # heads per core
DQ = HPC * HD       # 256: per-core projection width
ROPE = 64           # RoPE dims per head
F32 = mybir.dt.float32
F32R = mybir.dt.float32r
BF16 = mybir.dt.bfloat16
F8 = mybir.dt.float8e4
DR = mybir.MatmulPerfMode.DoubleRow

NT256 = L // 256    # 8 token chunks per batch for projections
NCHUNK = B * NT256  # 16 chunks in the linear stream
NI = L // 512       # 4 i-tiles per attention instance
NJ = L // 128       # 16 j-blocks

# fp8 scales: x*SX, w*SW quantized; product rescaled at PSUM drain
SX = 32.0
SWQ = 1024.0        # wq additionally carries 1/sqrt(HD)
SWK = 64.0
SWV = 64.0
QS = 1.0 / (SX * SWQ)
KS = 1.0 / (SX * SWK)
VS = 1.0 / (SX * SWV)
# att_sb holds SA*att (the softmax-denominator reciprocal is pre-scaled by
# SA via the ones constant) so its fp8 hi/lo split sits in normal range;
# wo is quantized at SWO. y-emit PSUM drains rescale by 1/(SA*SWO).
SA = 32.0
SWO = 64.0
YS8 = 1.0 / (SA * SWO)   # fp8 DoubleRow y-emit drain scale
YSB = 1.0 / SA           # bf16 (last tile) y-emit drain scale


def build_nc():
    nc = bacc.Bacc(
        "TRN2", target_bir_lowering=False, debug=False, num_devices=NCORES
    )
    xq = nc.dram_tensor("xq", [NCHUNK, 128, 16, 512], F8, kind="ExternalInput").ap()
    # weights packed hi|lo along the last axis: [:, :, 0:DQ]=hi, [DQ:2DQ]=lo
    wq8 = nc.dram_tensor("wq8", [128, 16, 2 * DQ], F8, kind="ExternalInput").ap()
    wk8 = nc.dram_tensor("wk8", [128, 16, 2 * DQ], F8, kind="ExternalInput").ap()
    wv8 = nc.dram_tensor("wv8", [128, 16, 2 * DQ], F8, kind="ExternalInput").ap()
    wo = nc.dram_tensor("wo", [128, HPC, D], BF16, kind="ExternalInput").ap()
    # cc rows = [cos;cos], ss rows = [-sin;+sin] (bf16), for pairs (i, i+32)
    cc = nc.dram_tensor("cc", [ROPE, L], BF16, kind="ExternalInput").ap()
    ss = nc.dram_tensor("ss", [ROPE, L], BF16, kind="ExternalInput").ap()
    m4 = nc.dram_tensor("m4", [128, 4, 512], BF16, kind="ExternalInput").ap()
    y = nc.dram_tensor("y", [B, L, D], BF16, kind="ExternalOutput").ap()

    with tile.TileContext(nc) as tc:
        with (
            tc.tile_pool(name="consts", bufs=1) as consts,
            tc.tile_pool(name="wpool", bufs=1) as wpool,
            tc.tile_pool(name="qkv", bufs=1) as qkv,
            tc.tile_pool(name="xc", bufs=4) as xcpool,
            tc.tile_pool(name="et", bufs=4) as etpool,
            tc.tile_pool(name="rope", bufs=4) as ropepool,
            tc.tile_pool(name="ysb", bufs=2) as ypool,
            tc.tile_pool(name="riv", bufs=2) as rivpool,
            tc.tile_pool(name="rr", bufs=2) as rrpool,
            tc.tile_pool(name="rbc", bufs=1) as rbcpool,
            tc.tile_pool(name="pA", bufs=3, space="PSUM") as pA,
            tc.tile_pool(name="pST", bufs=2, space="PSUM") as pST,
            tc.tile_pool(name="pAV", bufs=2, space="PSUM") as pAV,
            tc.tile_pool(name="pR", bufs=1, space="PSUM") as pR,
        ):
            # ---- constants (no DMA deps; built first so PE warm-up can run) ----
            # ones carries 1/SA so the denominator sums come out pre-scaled:
            # riv = SA/r and the normalized att tiles hold SA*att (fp8 range)
            ones = consts.tile([128, 1], BF16)
            nc.vector.memset(ones, 1.0 / SA)
            dummy = consts.tile([128, 512], BF16)
            nc.vector.memset(dummy, 0.0)
            ident_f = consts.tile([128, 128], F32)
            masks.make_identity(nc, ident_f)
            ident = consts.tile([128, 128], F32R)
            nc.vector.tensor_copy(ident, ident_f)

            # PE warm-up: cheap matmuls spanning the ~3us p-state ramp plus
            # the DMA-supply-bound stretch before the first chains unblock
            pwarm = pA.tile([128, 512], F32, tag="pA", name="pwarm")
            for _ in range(12):
                nc.tensor.matmul(
                    pwarm[0:1, :], (ones), (dummy), start=True, stop=True
                )

            # ---- weights + tables (order sets DMA_ENGINES priority) ----
            wq_sb = wpool.tile([128, 16, 2 * DQ], F8)
            wk_sb = wpool.tile([128, 16, 2 * DQ], F8)
            wv_sb = wpool.tile([128, 16, 2 * DQ], F8)
            wo_sb = wpool.tile([128, HPC, D], BF16)

            qt_sb = qkv.tile([128, HPC, L], BF16)   # [d, h, tok]
            kt_sb = qkv.tile([128, HPC, L], BF16)
            v_sb = qkv.tile([128, NJ, DQ], BF16)    # [tok_in_blk, jblk, hd]
            att_sb = qkv.tile([128, HPC, L], BF16)  # [hd, h, tok] = SA*att

            cc_sb = consts.tile([ROPE, L], BF16)
            ss_sb = consts.tile([ROPE, L], BF16)
            m4_sb = consts.tile([128, 4, 512], BF16)

            # ---- x chunk stream ----
            xc_tiles = {}

            def emit_xc_dma(i, half=None):
                # half=None: emit both halves, but only once per chunk;
                # half=0/1: priming-time single-half emission
                if i >= NCHUNK or (i in xc_tiles and half is None):
                    return
                if i in xc_tiles:
                    t_ = xc_tiles[i]
                else:
                    t_ = xcpool.tile([128, 16, 512], F8, tag="xc", name=f"xc{i}")
                    xc_tiles[i] = t_
                halves = range(2) if half is None else [half]
                for hf in halves:
                    nc.sync.dma_start(
                        out=t_[:, 8 * hf : 8 * hf + 8, :],
                        in_=xq[i, :, 8 * hf : 8 * hf + 8, :],
                    )

            # priming: DMA order matches the PE consumption order of the
            # first two chunks' chains so supply granularity stalls stay small
            nc.sync.dma_start(out=wq_sb, in_=wq8)
            emit_xc_dma(0, 0)
            emit_xc_dma(0, 1)
            nc.sync.dma_start(out=wk_sb, in_=wk8)
            emit_xc_dma(1, 0)
            emit_xc_dma(1, 1)
            emit_xc_dma(2)
            emit_xc_dma(3)
            nc.sync.dma_start(out=wv_sb, in_=wv8)
            nc.sync.dma_start(out=cc_sb, in_=cc)
            nc.sync.dma_start(out=ss_sb, in_=ss)
            nc.sync.dma_start(out=m4_sb, in_=m4)
            nc.sync.dma_start(out=wo_sb, in_=wo)

            # ---- y emission ----
            pending = []   # (b, t2, dd) poppable now
            staged = []    # (b, t2, dd) from the current tile, not yet poppable
            ycnt = [0]
            ysb_tiles = {}

            def emit_y(b, t2, dd):
                p = pA.tile([128, 512], F32, tag="pA", name=f"yp_{b}_{t2}_{dd}")
                tsl = slice(128 * t2, 128 * t2 + 128)
                dsl = slice(512 * dd, 512 * dd + 512)
                tail = b == B - 1 and t2 >= 4 * (NI - 1)
                nc.tensor.matmul(
                    p, (att_sb[:, 0, tsl]), (wo_sb[:, 0, dsl]),
                    start=True, stop=False,
                )
                nc.tensor.matmul(
                    p, (att_sb[:, 1, tsl]), (wo_sb[:, 1, dsl]),
                    start=False, stop=True,
                )
                key = (b, t2)
                if key not in ysb_tiles:
                    ysb_tiles[key] = ypool.tile(
                        [128, 2048], BF16, tag="ysb", name=f"y_{b}_{t2}"
                    )
                yt = ysb_tiles[key]
                dst = yt[:, 512 * dd : 512 * dd + 512]
                ysc = YSB
                final_slice = tail and t2 == 4 * NI - 1 and dd == 3
                if final_slice:
                    # the very last drain goes wholly to ACT: no waiting on
                    # the (laggier) DVE before the final DMA can launch
                    nc.scalar.activation(
                        dst, p, mybir.ActivationFunctionType.Copy, scale=ysc
                    )
                elif tail:
                    # drains are the tail throughput limit: split each across
                    # ACT and DVE so they keep pace with the matmuls
                    nc.scalar.activation(
                        dst[:, 0:256], p[:, 0:256],
                        mybir.ActivationFunctionType.Copy, scale=ysc,
                    )
                    nc.vector.tensor_scalar_mul(
                        dst[:, 256:512], p[:, 256:512], ysc
                    )
                elif ycnt[0] % 2 == 0:
                    nc.scalar.activation(
                        dst, p, mybir.ActivationFunctionType.Copy, scale=ysc
                    )
                else:
                    nc.vector.tensor_scalar_mul(dst, p, ysc)
                ycnt[0] += 1
                if tail and t2 == 4 * NI - 1:
                    # very last block: stream each 512-slice immediately on
                    # alternating HWDGE queues, minimizing the final chain
                    dq = nc.scalar if dd % 2 == 0 else nc.sync
                    dq.dma_start(
                        out=y[b, 128 * t2 : 128 * t2 + 128,
                              512 * dd : 512 * dd + 512],
                        in_=dst,
                    )
                    if dd == 3:
                        del ysb_tiles[key]
                elif tail:
                    # last tile: stream halves out early on two queues so the
                    # final writeback isn't one serialized chain at the end
                    if dd == 1:
                        nc.sync.dma_start(
                            out=y[b, 128 * t2 : 128 * t2 + 128, 0:1024],
                            in_=yt[:, 0:1024],
                        )
                    elif dd == 3:
                        nc.gpsimd.dma_start(
                            out=y[b, 128 * t2 : 128 * t2 + 128, 1024:2048],
                            in_=yt[:, 1024:2048],
                        )
                        del ysb_tiles[key]
                elif dd == 3:
                    nc.gpsimd.dma_start(
                        out=y[b, 128 * t2 : 128 * t2 + 128, :], in_=yt
                    )
                    del ysb_tiles[key]

            def pop_y():
                if pending:
                    emit_y(*pending.pop(0))

            # ---- projection units (fp8 DoubleRow, 3-term hi/lo residual) ----
            # terms: (hi x, hi w), (lo x, hi w), (hi x, lo w)
            def unit_qk_chain(i, w_sb, out_sb, rt, drain_act, scale):
                def run():
                    xc = xc_tiles[i]
                    tt = i % NT256
                    pp = pA.tile([128, 512], F32, tag="pA")
                    pj = pp[:, 0:256]
                    n = 0
                    for wo_, xo in ((0, 0), (0, 256), (DQ, 0)):
                        for c in range(8):
                            nc.tensor.matmul(
                                pj,
                                (w_sb[:, 2 * c : 2 * c + 2,
                                      wo_ + 128 * rt : wo_ + 128 * rt + 128]),
                                (xc[:, 2 * c : 2 * c + 2, xo : xo + 256]),
                                start=(n == 0),
                                stop=(n == 23),
                                perf_mode=DR,
                            )
                            n += 1
                    dst = out_sb[:, rt, 256 * tt : 256 * tt + 256]
                    if drain_act:
                        nc.scalar.activation(
                            dst, pj, mybir.ActivationFunctionType.Copy,
                            scale=scale,
                        )
                    else:
                        nc.vector.tensor_scalar_mul(dst, pj, scale)
                return run

            def unit_v_chain(i, ts2):
                def run():
                    xc = xc_tiles[i]
                    tt = i % NT256
                    pv = pA.tile([128, 512], F32, tag="pA")
                    pvj = pv[:, 0:256]
                    n = 0
                    for wo_, xo in ((0, 0), (0, 256), (DQ, 0)):
                        for c in range(8):
                            nc.tensor.matmul(
                                pvj,
                                (xc[:, 2 * c : 2 * c + 2,
                                    xo + 128 * ts2 : xo + 128 * ts2 + 128]),
                                (wv_sb[:, 2 * c : 2 * c + 2,
                                       wo_ : wo_ + DQ]),
                                start=(n == 0),
                                stop=(n == 23),
                                perf_mode=DR,
                            )
                            n += 1
                    nc.vector.tensor_scalar_mul(v_sb[:, 2 * tt + ts2, :], pvj, VS)
                return run

            # rot = [x1;x2]*[c;c] + [x2;x1]*[-s;s] on the 512-tok quarter.
            # Swap DMAs and muls are SEPARATE units: the tiny swap DMAs must
            # enter the DMA-engine FIFO before the 1.4us x-chunk prefetch
            # transfers of the same seg, or the muls (and the next tile's
            # first STs) stall ~3-4us behind them.
            rope_swaps = {}

            def unit_rope_swap(out_sb, rt, qq, key, dq):
                def run():
                    qsl = slice(512 * qq, 512 * qq + 512)
                    swap = ropepool.tile(
                        [ROPE, 512], BF16, tag="rope", name=f"sw{key}_{qq}"
                    )
                    rope_swaps[key] = swap
                    dq.dma_start(out=swap[0:32], in_=out_sb[32:64, rt, qsl])
                    dq.dma_start(out=swap[32:64], in_=out_sb[0:32, rt, qsl])
                return run

            def unit_rope_mul(out_sb, rt, qq, key):
                def run():
                    qsl = slice(512 * qq, 512 * qq + 512)
                    rope_rows = out_sb[0:ROPE, rt, qsl]
                    swap = rope_swaps.pop(key)
                    nc.vector.tensor_mul(swap, swap, ss_sb[:, qsl])
                    nc.vector.tensor_mul(rope_rows, rope_rows, cc_sb[:, qsl])
                    nc.vector.tensor_add(rope_rows, rope_rows, swap)
                return run

            def _qk4(i):
                return [
                    unit_qk_chain(i, wq_sb, qt_sb, 0, True, QS),
                    unit_qk_chain(i, wq_sb, qt_sb, 1, True, QS),
                    unit_qk_chain(i, wk_sb, kt_sb, 0, False, KS),
                    unit_qk_chain(i, wk_sb, kt_sb, 1, False, KS),
                ]

            def prefetch_units(i):
                # issued after the rope units: the 2.9us chunk DMAs must not
                # get ahead of the tiny rope-swap DMAs on the shared engines,
                # but still land before the NEXT seg's chains need them
                return [lambda: emit_xc_dma(i), lambda: emit_xc_dma(i + 1)]

            def chunk_v_units(i):
                return [unit_v_chain(i, 0), unit_v_chain(i, 1)]

            def rope_swap_units(qq):
                out = []
                for rt in range(HPC):
                    out.append(
                        unit_rope_swap(qt_sb, rt, qq, f"q{rt}", nc.scalar)
                    )
                    out.append(
                        unit_rope_swap(kt_sb, rt, qq, f"k{rt}", nc.scalar)
                    )
                return out

            def rope_mul_units(qq):
                out = []
                for rt in range(HPC):
                    out.append(unit_rope_mul(qt_sb, rt, qq, f"q{rt}"))
                    out.append(unit_rope_mul(kt_sb, rt, qq, f"k{rt}"))
                return out

            def rope_units(qq):
                return rope_swap_units(qq) + rope_mul_units(qq)

            # ---- attention ----
            def att_instance(b, h, t, seg, last=False, hold=0):
                # hold: skip pop_y on that many of every 2 j-steps, saving
                # pending y-emits to cover this/next instance's drain window
                av = pAV.tile([128, 512], F32, tag="pAV")
                # prt[:, 0:4] accumulates column sums (transposed denominators);
                # prt[0:1, :] is reused afterwards for the [1, 512] transpose
                prt = pR.tile([128, 512], F32, tag="pR")
                njb = 4 * t + 4
                for j in range(njb):
                    q = j - 4 * t
                    # exact causal widths: 512 / 384 / 256 / 128
                    off = 128 * q if q > 0 else 0
                    st = pST.tile([128, 512], F32, tag="pST")
                    nc.tensor.matmul(
                        st[:, off:512],
                        (kt_sb[:, h, 128 * j : 128 * j + 128]),
                        (qt_sb[:, h, 512 * t + off : 512 * t + 512]),
                        start=True,
                        stop=True,
                    )
                    et = etpool.tile([128, 512], BF16, tag="et")
                    nc.scalar.activation(
                        et[:, off:512],
                        st[:, off:512],
                        mybir.ActivationFunctionType.Exp,
                    )
                    if q >= 0:
                        # only cols [128q, 128q+128) are partially masked
                        # (beyond them causality always holds, mask==1)
                        msl = slice(128 * q, 128 * q + 128)
                        nc.vector.tensor_mul(
                            et[:, msl], et[:, msl], m4_sb[:, q, msl],
                        )
                    # fillers BEFORE rT/av: the ~4-deep OOO window parks on
                    # the exp->mask chain, so independent work must sit ahead
                    # of the parked dependents in program order
                    if seg:
                        seg.pop(0)()
                    if hold == 0 or (hold == 1 and j % 2 == 0):
                        pop_y()
                    # denominator column sums: et as stationary, out free = 1.
                    # PSUM pending-zero is 2KB-region granular, so only the
                    # very first matmul of the instance may carry start=True;
                    # later first-writes to other columns hit still-pending
                    # bytes and are zeroed-then-written by the hardware.
                    for ic in range(max(q, 0), 4):
                        nc.tensor.matmul(
                            prt[:, ic : ic + 1],
                            (et[:, 128 * ic : 128 * ic + 128]),
                            (ones),
                            start=(j == 0 and ic == 0),
                            stop=(j == 4 * t + ic),
                            skip_group_check=True,
                        )
                    nc.tensor.matmul(
                        av[:, off:512],
                        (v_sb[:, j, HD * h : HD * h + HD]),
                        (et[:, off:512]),
                        start=(j == 0),
                        stop=(j == njb - 1),
                        skip_group_check=True,
                    )
                # denominators: [128, 4] -> SBUF -> 4 PE transposes onto
                # partition 0 -> [1, 512] -> reciprocal -> partition broadcast
                rts = rivpool.tile([128, 4], F32R, tag="riv")
                nc.vector.tensor_copy(rts, prt[:, 0:4])
                # ready y-emit matmuls BEFORE the transposes: the transposes
                # stall ~0.5us on the rts copy and PE executes in order
                pop_y()
                pop_y()
                prt_r = prt.bitcast(F32R)
                for ic in range(4):
                    # start=True only on the first transpose: one pending-zero
                    # mark for partition 0's row, later ones zero-then-write
                    nc.tensor.matmul(
                        prt_r[0:1, 128 * ic : 128 * ic + 128],
                        rts[:, ic : ic + 1],
                        ident,
                        is_transpose=True,
                        start=(ic == 0),
                        stop=(ic == 3),
                        skip_group_check=True,
                    )
                if hold:
                    # held-back emits land here, filling the PE while the
                    # reciprocal/broadcast/normalize chain runs
                    pop_y()
                    pop_y()
                    pop_y()
                    pop_y()
                rbc = rbcpool.tile([128, 512], F32, tag="rbc")
                if last:
                    # split reciprocal/broadcast/normalize per quarter so the
                    # tail's first y tiles unblock as early as possible
                    riv = rrpool.tile([1, 512], F32, tag="rr")
                    for hf in range(4):
                        sl = slice(128 * hf, 128 * hf + 128)
                        nc.vector.reciprocal(riv[:, sl], prt[0:1, sl])
                        nc.gpsimd.partition_broadcast(rbc[:, sl], riv[:, sl])
                        nc.vector.tensor_mul(
                            att_sb[:, h, 512 * t + 128 * hf :
                                   512 * t + 128 * hf + 128],
                            av[:, sl], rbc[:, sl],
                        )
                        if hf > 0:
                            pop_y()
                else:
                    riv = rrpool.tile([1, 512], F32, tag="rr")
                    nc.vector.reciprocal(riv, prt[0:1, 0:512])
                    nc.gpsimd.partition_broadcast(rbc, riv)
                    sl_t = slice(512 * t, 512 * t + 512)
                    nc.vector.tensor_mul(att_sb[:, h, sl_t], av, rbc)

            # ---------------- main schedule ----------------
            # The qk chains for quarter q run one attention-tile EARLIER than
            # quarter q's attention (chains for q=0,1 in the prologue, chains
            # for q+1 in seg of tile q-1... i.e. seg_t carries chains(t+2)),
            # so rope units for tile t+1 sit at the HEAD of seg_t with their
            # inputs already drained — their swap-DMA + mul latency hides
            # under a whole tile of attention instead of stalling tile t+1.
            for b in range(B):
                base = NT256 * b
                emit_xc_dma(base)
                emit_xc_dma(base + 1)
                pro = _qk4(base) + _qk4(base + 1) + _qk4(base + 2) + _qk4(
                    base + 3
                )
                pro_pop = (
                    rope_units(0)
                    + chunk_v_units(base)
                    + chunk_v_units(base + 1)
                    + prefetch_units(base + 4)
                )
                if b > 0:
                    pro_pop = prefetch_units(base + 2) + pro_pop
                for u in pro:
                    u()
                pending.extend(staged)
                staged.clear()
                for u in pro_pop:
                    u()
                    pop_y()
                    pop_y()
                for t in range(NI):
                    # emits staged by tile t-1 become poppable now: their
                    # att8 prep races only the first few j-steps (gated)
                    pending.extend(staged)
                    staged.clear()
                    if t < NI - 2:
                        seg = (
                            rope_units(t + 1)
                            + _qk4(base + 2 * t + 4)
                            + _qk4(base + 2 * t + 5)
                            + prefetch_units(base + 2 * t + 6)
                            + chunk_v_units(base + 2 * t + 2)
                            + chunk_v_units(base + 2 * t + 3)
                        )
                    elif t == NI - 2:
                        seg = (
                            rope_units(t + 1)
                            + prefetch_units(base + 2 * t + 6)
                            + chunk_v_units(base + 2 * t + 2)
                            + chunk_v_units(base + 2 * t + 3)
                        )
                    else:
                        seg = []
                    final_tile = b == B - 1 and t == NI - 1
                    # hold pops on every batch's last tile: its seg is empty,
                    # so saved emits are the only PE filler for the drain
                    # windows and the following prologue/tail boundary
                    batch_last = t == NI - 1
                    for h in range(HPC):
                        att_instance(
                            b, h, t, seg,
                            last=(final_tile and h == HPC - 1),
                            hold=(h + 1 if batch_last else 0),
                        )
                    staged.extend(
                        (b, t2, dd)
                        for t2 in range(4 * t, 4 * t + 4)
                        for dd in range(4)
                    )
                    while seg:
                        seg.pop(0)()
            pending.extend(staged)
            staged.clear()
            while pending:
                emit_y(*pending.pop(0))
    nc.compile()
    return nc


_NC = None


def _get_nc():
    global _NC
    if _NC is None:
        _NC = build_nc()
    return _NC


def _fp8_split(a, s):
    import ml_dtypes

    f8 = ml_dtypes.float8_e4m3fn
    hi = (a * np.float32(s)).astype(f8)
    lo = ((a * np.float32(s)) - hi.astype(np.float32)).astype(f8)
    return hi, lo


def _host_inputs(x, mask, wq, wk, wv, wo):
    import ml_dtypes

    x = np.asarray(x, np.float32)
    wq = np.asarray(wq, np.float32)
    wk = np.asarray(wk, np.float32)
    wv = np.asarray(wv, np.float32)
    wo = np.asarray(wo, np.float32)

    # x chunks: [chunk, p, c, hi(256)|lo(256)] fp8, d = 128c + p
    xr = (
        x.transpose(0, 2, 1)          # [B, D, L]
        .reshape(B, 16, 128, NT256, 256)
        .transpose(0, 3, 2, 1, 4)     # [b, tt, p, c, tok]
    )
    xh, xl = _fp8_split(xr, SX)
    xq = np.ascontiguousarray(
        np.concatenate([xh, xl], axis=-1).reshape(NCHUNK, 128, 16, 512)
    )

    # permute head dims so RoPE pairs are (i, i+32): [evens, odds, pass]
    perm128 = np.concatenate(
        [np.arange(0, ROPE, 2), np.arange(1, ROPE, 2), np.arange(ROPE, HD)]
    )
    permD = np.concatenate([h * HD + perm128 for h in range(H)])
    wq_p = (wq * np.float32(1.0 / np.sqrt(HD)))[:, permD]
    wk_p = wk[:, permD]

    def wtile(w2d):  # [D, DQ] -> [128 p, 16 c, DQ]
        return np.ascontiguousarray(
            w2d.reshape(16, 128, DQ).transpose(1, 0, 2)
        )

    # RoPE tables, matching reference fp32 math (dim=64, repeat-2 interleave)
    ts_ = np.arange(0, ROPE, 2, dtype=np.float32)
    inv = (np.float32(10000.0) ** (-ts_ / np.float32(ROPE))).astype(np.float32)
    grid = np.arange(L, dtype=np.float32)[:, None] * inv[None, :]  # [L, 32]
    ccm = np.empty((ROPE, L), np.float32)
    ccm[0:32] = ccm[32:64] = np.cos(grid).T
    ssm = np.empty((ROPE, L), np.float32)
    ssm[0:32] = -np.sin(grid).T
    ssm[32:64] = np.sin(grid).T
    ccm = ccm.astype(ml_dtypes.bfloat16)
    ssm = ssm.astype(ml_dtypes.bfloat16)

    # diagonal-quarter masks for ST tiles: m4[j, q, i] = mask[i, 128q + j]
    mm = np.asarray(mask[0, 0, :512, :512])
    m4 = (
        mm.T.reshape(4, 128, 512).transpose(1, 0, 2).astype(ml_dtypes.bfloat16)
    )  # [j, q, i]

    in_maps = []
    for c in range(NCORES):
        sl = slice(DQ * c, DQ * c + DQ)
        def pack_hl(w2d, s):
            hi, lo = _fp8_split(wtile(w2d), s)
            return np.ascontiguousarray(np.concatenate([hi, lo], axis=-1))

        wq8_ = pack_hl(wq_p[:, sl], SWQ)
        wk8_ = pack_hl(wk_p[:, sl], SWK)
        wv8_ = pack_hl(wv[:, sl], SWV)
        wo_t = np.ascontiguousarray(
            wo[sl, :].reshape(HPC, HD, D).transpose(1, 0, 2)
        ).astype(ml_dtypes.bfloat16)
        in_maps.append(
            {
                "xq": xq,
                "wq8": wq8_, "wk8": wk8_, "wv8": wv8_,
                "wo": wo_t,
                "cc": ccm,
                "ss": ssm,
                "m4": m4,
            }
        )
    return in_maps


def _reference_host(x, mask, wq, wk, wv, wo):
    """Exact-math numpy fallback (used only if the mask is not causal-tril)."""
    Hh, P = H, 64
    xx = np.asarray(x, np.float32)
    Bb, Ll, Dd = xx.shape
    K = Dd // Hh

    def rope(t):  # [b,h,s,d]
        d, s = t.shape[-1], t.shape[-2]
        ts_ = np.arange(0, d, 2, dtype=np.float32)
        inv = (np.float32(10000.0) ** (-ts_ / np.float32(d)))
        grid = np.arange(s, dtype=np.float32)[:, None] * inv[None, :]
        sin = np.repeat(np.sin(grid), 2, axis=-1)[None, None]
        cos = np.repeat(np.cos(grid), 2, axis=-1)[None, None]
        x1, x2 = t[..., ::2], t[..., 1::2]
        xs = np.stack([-x2, x1], axis=-1).reshape(t.shape)
        return t * cos + xs * sin

    def split(t):
        return t.reshape(Bb, Ll, Hh, K).transpose(0, 2, 1, 3)

    q = split(xx @ np.asarray(wq, np.float32)) / np.sqrt(K)
    q = np.concatenate([rope(q[..., :P]), q[..., P:]], axis=-1)
    k = split(xx @ np.asarray(wk, np.float32))
    k = np.concatenate([rope(k[..., :P]), k[..., P:]], axis=-1)
    v = split(xx @ np.asarray(wv, np.float32))
    s = np.einsum("bhik,bhjk->bhij", q, k)
    s = np.where(np.asarray(mask), s, np.float32(-1e8))
    s -= s.max(axis=-1, keepdims=True)
    e = np.exp(s)
    a = e / e.sum(axis=-1, keepdims=True)
    yy = np.einsum("bhij,bhjv->bhiv", a, v)
    yy = yy.transpose(0, 2, 1, 3).reshape(Bb, Ll, Dd)
    return (yy @ np.asarray(wo, np.float32)).astype(np.float32)


def kernel(**inputs):
    mask_arr = np.asarray(inputs["mask"])
    if not bool(
        (mask_arr[0, 0] == np.tril(np.ones((L, L), bool))).all()
    ):
        return _reference_host(
            inputs["x"], inputs["mask"], inputs["wq"], inputs["wk"],
            inputs["wv"], inputs["wo"],
        )
    nc = _get_nc()
    in_maps = _host_inputs(
        inputs["x"], inputs["mask"], inputs["wq"], inputs["wk"],
        inputs["wv"], inputs["wo"],
    )
    res = run_bass_kernel_spmd(nc, in_maps, core_ids=list(range(NCORES)))
    out = res.results[0]["y"].astype(np.float64)
    for c in range(1, NCORES):
        out += res.results[c]["y"]
    return out.astype(np.float32)


# revision 85
# speedup vs baseline: 1.2891x; 1.0040x over previous
"""Multi-head attention (B=2, L=2048, D=2048, H=16, causal + RoPE) on 8 TRN2 cores.

Sharding: tensor-parallel over heads. Core c owns heads {2c, 2c+1}:
  - wq/wk/wv column slices [D, 256], wo row slice [256, D]
  - each core computes a partial output y_c = att_c @ wo_c  (full shape)
  - host reduces: y = sum_c y_c   (the "all-reduce" of the output projection)

v3 schedule (vs v2 baseline):
  - QKV projections in fp8(e4m3) DoubleRow with a hi/lo residual split:
    x = (xh + xl)/SX, w = (wh + wl)/SW; x@w ~ (xh@wh + xl@wh + xh@wl)
    / (SX*SW).  3 DoubleRow chains of 8 MMs replace 16 fp32r MMs per
    256-token chain (24*128 vs 16*256 PE cycles).
  - Softmax denominators via et-as-STATIONARY matmuls: out [128i, 1]
    per 128-col slice of et (free-dim cost 1 instead of width), then a
    PE transpose of the accumulated [128,4] into [1,512], reciprocal,
    partition-broadcast, and the usual normalize-on-drain.
  - qt/kt/v/et/att all bf16: halves DVE element cost where 2x modes
    apply and lifts the fp32r >=256 moving-width restriction, so
    diagonal score blocks shrink to exact causal widths 512/384/256/128.
  - x chunks DMA'd as fp8 hi|lo packed [128,16,512] tiles (2 DMAs per
    chunk instead of 8, half the bytes of f32).
  - y written back per 128-token block [128, 2048] (1 DMA instead of 4).
  - PE warm-up: a few dummy matmuls at t~0 so the p-state ramp finishes
    before the first real projection chain.
"""

import glob
import os


def _ensure_env():
    # walrus_driver (neuronx-cc) must be on PATH for client-side NEFF compile.
    if not any("-b16-bazel-" in p for p in os.environ.get("PATH", "").split(":")):
        cands = sorted(glob.glob("/nix/store/*-b16-bazel-*/bin"))
        for c in cands:
            if os.path.exists(os.path.join(c, "neuronx-cc")) or glob.glob(
                os.path.join(c, "*walrus*")
            ):
                os.environ["PATH"] = c + ":" + os.environ["PATH"]
                break
        else:
            if cands:
                os.environ["PATH"] = cands[-1] + ":" + os.environ["PATH"]


_ensure_env()
os.environ.setdefault("JAX_COMPILATION_CACHE_DIR", "/tmp/jax_comp_cache")
os.environ.setdefault("JAX_PERSISTENT_CACHE_MIN_COMPILE_TIME_SECS", "1")
os.environ.setdefault("JAX_PERSISTENT_CACHE_MIN_ENTRY_SIZE_BYTES", "0")

import numpy as np  # noqa: E402

import concourse.bass as bass  # noqa: E402
import concourse.mybir as mybir  # noqa: E402
import concourse.tile as tile  # noqa: E402
from concourse import bacc, masks  # noqa: E402
from concourse.bass_utils import run_bass_kernel_spmd  # noqa: E402

NCORES = 8
B, L, D = 2, 2048, 2048
H = 16
HD = 128            # head dim
HPC = H // NCORES   # heads per core
DQ = HPC * HD       # 256: per-core projection width
ROPE = 64           # RoPE dims per head
F32 = mybir.dt.float32
F32R = mybir.dt.float32r
BF16 = mybir.dt.bfloat16
F8 = mybir.dt.float8e4
DR = mybir.MatmulPerfMode.DoubleRow

NT256 = L // 256    # 8 token chunks per batch for projections
NCHUNK = B * NT256  # 16 chunks in the linear stream
NI = L // 512       # 4 i-tiles per attention instance
NJ = L // 128       # 16 j-blocks

# fp8 scales: x*SX, w*SW quantized; product rescaled at PSUM drain
SX = 32.0
SWQ = 1024.0        # wq additionally carries 1/sqrt(HD)
SWK = 64.0
SWV = 64.0
QS = 1.0 / (SX * SWQ)
KS = 1.0 / (SX * SWK)
VS = 1.0 / (SX * SWV)
# att_sb holds SA*att (the softmax-denominator reciprocal is pre-scaled by
# SA via the ones constant) so its fp8 hi/lo split sits in normal range;
# wo is quantized at SWO. y-emit PSUM drains rescale by 1/(SA*SWO).
SA = 32.0
SWO = 64.0
YS8 = 1.0 / (SA * SWO)   # fp8 DoubleRow y-emit drain scale
YSB = 1.0 / SA           # bf16 (last tile) y-emit drain scale


def build_nc():
    nc = bacc.Bacc(
        "TRN2", target_bir_lowering=False, debug=False, num_devices=NCORES
    )
    xq = nc.dram_tensor("xq", [NCHUNK, 128, 16, 512], F8, kind="ExternalInput").ap()
    # weights packed hi|lo along the last axis: [:, :, 0:DQ]=hi, [DQ:2DQ]=lo
    wq8 = nc.dram_tensor("wq8", [128, 16, 2 * DQ], F8, kind="ExternalInput").ap()
    wk8 = nc.dram_tensor("wk8", [128, 16, 2 * DQ], F8, kind="ExternalInput").ap()
    wv8 = nc.dram_tensor("wv8", [128, 16, 2 * DQ], F8, kind="ExternalInput").ap()
    wo = nc.dram_tensor("wo", [128, HPC, D], BF16, kind="ExternalInput").ap()
    # cc rows = [cos;cos], ss rows = [-sin;+sin] (bf16), for pairs (i, i+32)
    cc = nc.dram_tensor("cc", [ROPE, L], BF16, kind="ExternalInput").ap()
    ss = nc.dram_tensor("ss", [ROPE, L], BF16, kind="ExternalInput").ap()
    m4 = nc.dram_tensor("m4", [128, 4, 512], BF16, kind="ExternalInput").ap()
    y = nc.dram_tensor("y", [B, L, D], BF16, kind="ExternalOutput").ap()

    with tile.TileContext(nc) as tc:
        with (
            tc.tile_pool(name="consts", bufs=1) as consts,
            tc.tile_pool(name="wpool", bufs=1) as wpool,
            tc.tile_pool(name="qkv", bufs=1) as qkv,
            tc.tile_pool(name="xc", bufs=4) as xcpool,
            tc.tile_pool(name="et", bufs=4) as etpool,
            tc.tile_pool(name="rope", bufs=4) as ropepool,
            tc.tile_pool(name="ysb", bufs=2) as ypool,
            tc.tile_pool(name="riv", bufs=2) as rivpool,
            tc.tile_pool(name="rr", bufs=2) as rrpool,
            tc.tile_pool(name="rbc", bufs=1) as rbcpool,
            tc.tile_pool(name="pA", bufs=3, space="PSUM") as pA,
            tc.tile_pool(name="pST", bufs=2, space="PSUM") as pST,
            tc.tile_pool(name="pAV", bufs=2, space="PSUM") as pAV,
            tc.tile_pool(name="pR", bufs=1, space="PSUM") as pR,
        ):
            # ---- constants (no DMA deps; built first so PE warm-up can run) ----
            # ones carries 1/SA so the denominator sums come out pre-scaled:
            # riv = SA/r and the normalized att tiles hold SA*att (fp8 range)
            ones = consts.tile([128, 1], BF16)
            nc.vector.memset(ones, 1.0 / SA)
            dummy = consts.tile([128, 512], BF16)
            nc.vector.memset(dummy, 0.0)
            ident_f = consts.tile([128, 128], F32)
            masks.make_identity(nc, ident_f)
            ident = consts.tile([128, 128], F32R)
            nc.vector.tensor_copy(ident, ident_f)

            # PE warm-up: cheap matmuls spanning the ~3us p-state ramp plus
            # the DMA-supply-bound stretch before the first chains unblock
            pwarm = pA.tile([128, 512], F32, tag="pA", name="pwarm")
            for _ in range(12):
                nc.tensor.matmul(
                    pwarm[0:1, :], (ones), (dummy), start=True, stop=True
                )

            # ---- weights + tables (order sets DMA_ENGINES priority) ----
            wq_sb = wpool.tile([128, 16, 2 * DQ], F8)
            wk_sb = wpool.tile([128, 16, 2 * DQ], F8)
            wv_sb = wpool.tile([128, 16, 2 * DQ], F8)
            wo_sb = wpool.tile([128, HPC, D], BF16)

            qt_sb = qkv.tile([128, HPC, L], BF16)   # [d, h, tok]
            kt_sb = qkv.tile([128, HPC, L], BF16)
            v_sb = qkv.tile([128, NJ, DQ], BF16)    # [tok_in_blk, jblk, hd]
            att_sb = qkv.tile([128, HPC, L], BF16)  # [hd, h, tok] = SA*att

            cc_sb = consts.tile([ROPE, L], BF16)
            ss_sb = consts.tile([ROPE, L], BF16)
            m4_sb = consts.tile([128, 4, 512], BF16)

            # ---- x chunk stream ----
            xc_tiles = {}

            def emit_xc_dma(i, half=None):
                # half=None: emit both halves, but only once per chunk;
                # half=0/1: priming-time single-half emission
                if i >= NCHUNK or (i in xc_tiles and half is None):
                    return
                if i in xc_tiles:
                    t_ = xc_tiles[i]
                else:
                    t_ = xcpool.tile([128, 16, 512], F8, tag="xc", name=f"xc{i}")
                    xc_tiles[i] = t_
                halves = range(2) if half is None else [half]
                for hf in halves:
                    nc.sync.dma_start(
                        out=t_[:, 8 * hf : 8 * hf + 8, :],
                        in_=xq[i, :, 8 * hf : 8 * hf + 8, :],
                    )

            # priming: DMA order matches the PE consumption order of the
            # first two chunks' chains so supply granularity stalls stay small
            nc.sync.dma_start(out=wq_sb, in_=wq8)
            emit_xc_dma(0, 0)
            emit_xc_dma(0, 1)
            nc.sync.dma_start(out=wk_sb, in_=wk8)
            emit_xc_dma(1, 0)
            emit_xc_dma(1, 1)
            emit_xc_dma(2)
            emit_xc_dma(3)
            nc.sync.dma_start(out=wv_sb, in_=wv8)
            nc.sync.dma_start(out=cc_sb, in_=cc)
            nc.sync.dma_start(out=ss_sb, in_=ss)
            nc.sync.dma_start(out=m4_sb, in_=m4)
            nc.sync.dma_start(out=wo_sb, in_=wo)

            # ---- y emission ----
            pending = []   # (b, t2, dd) poppable now
            staged = []    # (b, t2, dd) from the current tile, not yet poppable
            ycnt = [0]
            ysb_tiles = {}

            def emit_y(b, t2, dd):
                p = pA.tile([128, 512], F32, tag="pA", name=f"yp_{b}_{t2}_{dd}")
                tsl = slice(128 * t2, 128 * t2 + 128)
                dsl = slice(512 * dd, 512 * dd + 512)
                tail = b == B - 1 and t2 >= 4 * (NI - 1)
                nc.tensor.matmul(
                    p, (att_sb[:, 0, tsl]), (wo_sb[:, 0, dsl]),
                    start=True, stop=False,
                )
                nc.tensor.matmul(
                    p, (att_sb[:, 1, tsl]), (wo_sb[:, 1, dsl]),
                    start=False, stop=True,
                )
                key = (b, t2)
                if key not in ysb_tiles:
                    ysb_tiles[key] = ypool.tile(
                        [128, 2048], BF16, tag="ysb", name=f"y_{b}_{t2}"
                    )
                yt = ysb_tiles[key]
                dst = yt[:, 512 * dd : 512 * dd + 512]
                ysc = YSB
                final_slice = tail and t2 == 4 * NI - 1 and dd == 3
                if final_slice:
                    # the very last drain goes wholly to ACT: no waiting on
                    # the (laggier) DVE before the final DMA can launch
                    nc.scalar.activation(
                        dst, p, mybir.ActivationFunctionType.Copy, scale=ysc
                    )
                elif tail:
                    # drains are the tail throughput limit: split each across
                    # ACT and DVE so they keep pace with the matmuls
                    nc.scalar.activation(
                        dst[:, 0:256], p[:, 0:256],
                        mybir.ActivationFunctionType.Copy, scale=ysc,
                    )
                    nc.vector.tensor_scalar_mul(
                        dst[:, 256:512], p[:, 256:512], ysc
                    )
                elif ycnt[0] % 2 == 0:
                    nc.scalar.activation(
                        dst, p, mybir.ActivationFunctionType.Copy, scale=ysc
                    )
                else:
                    nc.vector.tensor_scalar_mul(dst, p, ysc)
                ycnt[0] += 1
                if tail and t2 == 4 * NI - 1:
                    # very last block: stream each 512-slice immediately on
                    # alternating HWDGE queues, minimizing the final chain
                    dq = nc.scalar if dd % 2 == 0 else nc.sync
                    dq.dma_start(
                        out=y[b, 128 * t2 : 128 * t2 + 128,
                              512 * dd : 512 * dd + 512],
                        in_=dst,
                    )
                    if dd == 3:
                        del ysb_tiles[key]
                elif tail:
                    # last tile: stream halves out early on two queues so the
                    # final writeback isn't one serialized chain at the end
                    if dd == 1:
                        nc.sync.dma_start(
                            out=y[b, 128 * t2 : 128 * t2 + 128, 0:1024],
                            in_=yt[:, 0:1024],
                        )
                    elif dd == 3:
                        nc.gpsimd.dma_start(
                            out=y[b, 128 * t2 : 128 * t2 + 128, 1024:2048],
                            in_=yt[:, 1024:2048],
                        )
                        del ysb_tiles[key]
                elif dd == 3:
                    nc.gpsimd.dma_start(
                        out=y[b, 128 * t2 : 128 * t2 + 128, :], in_=yt
                    )
                    del ysb_tiles[key]

            def pop_y():
                if pending:
                    emit_y(*pending.pop(0))

            # ---- projection units (fp8 DoubleRow, 3-term hi/lo residual) ----
            # terms: (hi x, hi w), (lo x, hi w), (hi x, lo w)
            def unit_qk_chain(i, w_sb, out_sb, rt, drain_act, scale):
                def run():
                    xc = xc_tiles[i]
                    tt = i % NT256
                    pp = pA.tile([128, 512], F32, tag="pA")
                    pj = pp[:, 0:256]
                    n = 0
                    for wo_, xo in ((0, 0), (0, 256), (DQ, 0)):
                        for c in range(8):
                            nc.tensor.matmul(
                                pj,
                                (w_sb[:, 2 * c : 2 * c + 2,
                                      wo_ + 128 * rt : wo_ + 128 * rt + 128]),
                                (xc[:, 2 * c : 2 * c + 2, xo : xo + 256]),
                                start=(n == 0),
                                stop=(n == 23),
                                perf_mode=DR,
                            )
                            n += 1
                    dst = out_sb[:, rt, 256 * tt : 256 * tt + 256]
                    if drain_act:
                        nc.scalar.activation(
                            dst, pj, mybir.ActivationFunctionType.Copy,
                            scale=scale,
                        )
                    else:
                        nc.vector.tensor_scalar_mul(dst, pj, scale)
                return run

            def unit_v_chain(i, ts2):
                def run():
                    xc = xc_tiles[i]
                    tt = i % NT256
                    pv = pA.tile([128, 512], F32, tag="pA")
                    pvj = pv[:, 0:256]
                    n = 0
                    for wo_, xo in ((0, 0), (0, 256), (DQ, 0)):
                        for c in range(8):
                            nc.tensor.matmul(
                                pvj,
                                (xc[:, 2 * c : 2 * c + 2,
                                    xo + 128 * ts2 : xo + 128 * ts2 + 128]),
                                (wv_sb[:, 2 * c : 2 * c + 2,
                                       wo_ : wo_ + DQ]),
                                start=(n == 0),
                                stop=(n == 23),
                                perf_mode=DR,
                            )
                            n += 1
                    nc.vector.tensor_scalar_mul(v_sb[:, 2 * tt + ts2, :], pvj, VS)
                return run

            # rot = [x1;x2]*[c;c] + [x2;x1]*[-s;s] on the 512-tok quarter.
            # Swap DMAs and muls are SEPARATE units: the tiny swap DMAs must
            # enter the DMA-engine FIFO before the 1.4us x-chunk prefetch
            # transfers of the same seg, or the muls (and the next tile's
            # first STs) stall ~3-4us behind them.
            rope_swaps = {}

            def unit_rope_swap(out_sb, rt, qq, key, dq):
                def run():
                    qsl = slice(512 * qq, 512 * qq + 512)
                    swap = ropepool.tile(
                        [ROPE, 512], BF16, tag="rope", name=f"sw{key}_{qq}"
                    )
                    rope_swaps[key] = swap
                    dq.dma_start(out=swap[0:32], in_=out_sb[32:64, rt, qsl])
                    dq.dma_start(out=swap[32:64], in_=out_sb[0:32, rt, qsl])
                return run

            def unit_rope_mul(out_sb, rt, qq, key):
                def run():
                    qsl = slice(512 * qq, 512 * qq + 512)
                    rope_rows = out_sb[0:ROPE, rt, qsl]
                    swap = rope_swaps.pop(key)
                    nc.vector.tensor_mul(swap, swap, ss_sb[:, qsl])
                    nc.vector.tensor_mul(rope_rows, rope_rows, cc_sb[:, qsl])
                    nc.vector.tensor_add(rope_rows, rope_rows, swap)
                return run

            def _qk4(i):
                return [
                    unit_qk_chain(i, wq_sb, qt_sb, 0, True, QS),
                    unit_qk_chain(i, wq_sb, qt_sb, 1, True, QS),
                    unit_qk_chain(i, wk_sb, kt_sb, 0, False, KS),
                    unit_qk_chain(i, wk_sb, kt_sb, 1, False, KS),
                ]

            def prefetch_units(i):
                # issued after the rope units: the 2.9us chunk DMAs must not
                # get ahead of the tiny rope-swap DMAs on the shared engines,
                # but still land before the NEXT seg's chains need them
                return [lambda: emit_xc_dma(i), lambda: emit_xc_dma(i + 1)]

            def chunk_v_units(i):
                return [unit_v_chain(i, 0), unit_v_chain(i, 1)]

            def rope_swap_units(qq):
                out = []
                for rt in range(HPC):
                    out.append(
                        unit_rope_swap(qt_sb, rt, qq, f"q{rt}", nc.scalar)
                    )
                    out.append(
                        unit_rope_swap(kt_sb, rt, qq, f"k{rt}", nc.scalar)
                    )
                return out

            def rope_mul_units(qq):
                out = []
                for rt in range(HPC):
                    out.append(unit_rope_mul(qt_sb, rt, qq, f"q{rt}"))
                    out.append(unit_rope_mul(kt_sb, rt, qq, f"k{rt}"))
                return out

            def rope_units(qq):
                return rope_swap_units(qq) + rope_mul_units(qq)

            # ---- attention ----
            def att_instance(b, h, t, seg, last=False, hold=0):
                # hold: skip pop_y on that many of every 2 j-steps, saving
                # pending y-emits to cover this/next instance's drain window
                av = pAV.tile([128, 512], F32, tag="pAV")
                # prt[:, 0:4] accumulates column sums (transposed denominators);
                # prt[0:1, :] is reused afterwards for the [1, 512] transpose
                prt = pR.tile([128, 512], F32, tag="pR")
                njb = 4 * t + 4
                for j in range(njb):
                    q = j - 4 * t
                    # exact causal widths: 512 / 384 / 256 / 128
                    off = 128 * q if q > 0 else 0
                    st = pST.tile([128, 512], F32, tag="pST")
                    nc.tensor.matmul(
                        st[:, off:512],
                        (kt_sb[:, h, 128 * j : 128 * j + 128]),
                        (qt_sb[:, h, 512 * t + off : 512 * t + 512]),
                        start=True,
                        stop=True,
                    )
                    et = etpool.tile([128, 512], BF16, tag="et")
                    nc.scalar.activation(
                        et[:, off:512],
                        st[:, off:512],
                        mybir.ActivationFunctionType.Exp,
                    )
                    if q >= 0:
                        # only cols [128q, 128q+128) are partially masked
                        # (beyond them causality always holds, mask==1)
                        msl = slice(128 * q, 128 * q + 128)
                        nc.vector.tensor_mul(
                            et[:, msl], et[:, msl], m4_sb[:, q, msl],
                        )
                    # fillers BEFORE rT/av: the ~4-deep OOO window parks on
                    # the exp->mask chain, so independent work must sit ahead
                    # of the parked dependents in program order
                    if seg:
                        seg.pop(0)()
                    if hold == 0 or (hold == 1 and j % 4 == 0):
                        pop_y()
                    # denominator column sums: et as stationary, out free = 1.
                    # PSUM pending-zero is 2KB-region granular, so only the
                    # very first matmul of the instance may carry start=True;
                    # later first-writes to other columns hit still-pending
                    # bytes and are zeroed-then-written by the hardware.
                    for ic in range(max(q, 0), 4):
                        nc.tensor.matmul(
                            prt[:, ic : ic + 1],
                            (et[:, 128 * ic : 128 * ic + 128]),
                            (ones),
                            start=(j == 0 and ic == 0),
                            stop=(j == 4 * t + ic),
                            skip_group_check=True,
                        )
                    nc.tensor.matmul(
                        av[:, off:512],
                        (v_sb[:, j, HD * h : HD * h + HD]),
                        (et[:, off:512]),
                        start=(j == 0),
                        stop=(j == njb - 1),
                        skip_group_check=True,
                    )
                # denominators: [128, 4] -> SBUF -> 4 PE transposes onto
                # partition 0 -> [1, 512] -> reciprocal -> partition broadcast
                rts = rivpool.tile([128, 4], F32R, tag="riv")
                nc.vector.tensor_copy(rts, prt[:, 0:4])
                # ready y-emit matmuls BEFORE the transposes: the transposes
                # stall ~0.5us on the rts copy and PE executes in order
                pop_y()
                pop_y()
                prt_r = prt.bitcast(F32R)
                for ic in range(4):
                    # start=True only on the first transpose: one pending-zero
                    # mark for partition 0's row, later ones zero-then-write
                    nc.tensor.matmul(
                        prt_r[0:1, 128 * ic : 128 * ic + 128],
                        rts[:, ic : ic + 1],
                        ident,
                        is_transpose=True,
                        start=(ic == 0),
                        stop=(ic == 3),
                        skip_group_check=True,
                    )
                if hold:
                    # held-back emits land here, filling the PE while the
                    # reciprocal/broadcast/normalize chain runs
                    for _ in range(4):
                        pop_y()
                rbc = rbcpool.tile([128, 512], F32, tag="rbc")
                if last:
                    # split reciprocal/broadcast/normalize per quarter so the
                    # tail's first y tiles unblock as early as possible
                    riv = rrpool.tile([1, 512], F32, tag="rr")
                    for hf in range(4):
                        sl = slice(128 * hf, 128 * hf + 128)
                        nc.vector.reciprocal(riv[:, sl], prt[0:1, sl])
                        nc.gpsimd.partition_broadcast(rbc[:, sl], riv[:, sl])
                        nc.vector.tensor_mul(
                            att_sb[:, h, 512 * t + 128 * hf :
                                   512 * t + 128 * hf + 128],
                            av[:, sl], rbc[:, sl],
                        )
                        if hf > 0:
                            pop_y()
                else:
                    riv = rrpool.tile([1, 512], F32, tag="rr")
                    nc.vector.reciprocal(riv, prt[0:1, 0:512])
                    nc.gpsimd.partition_broadcast(rbc, riv)
                    sl_t = slice(512 * t, 512 * t + 512)
                    nc.vector.tensor_mul(att_sb[:, h, sl_t], av, rbc)

            # ---------------- main schedule ----------------
            # The qk chains for quarter q run one attention-tile EARLIER than
            # quarter q's attention (chains for q=0,1 in the prologue, chains
            # for q+1 in seg of tile q-1... i.e. seg_t carries chains(t+2)),
            # so rope units for tile t+1 sit at the HEAD of seg_t with their
            # inputs already drained — their swap-DMA + mul latency hides
            # under a whole tile of attention instead of stalling tile t+1.
            for b in range(B):
                base = NT256 * b
                emit_xc_dma(base)
                emit_xc_dma(base + 1)
                pro = _qk4(base) + _qk4(base + 1) + _qk4(base + 2) + _qk4(
                    base + 3
                )
                pro_pop = (
                    rope_units(0)
                    + chunk_v_units(base)
                    + chunk_v_units(base + 1)
                    + prefetch_units(base + 4)
                )
                if b > 0:
                    pro_pop = prefetch_units(base + 2) + pro_pop
                for u in pro:
                    u()
                pending.extend(staged)
                staged.clear()
                for u in pro_pop:
                    u()
                    pop_y()
                    pop_y()
                for t in range(NI):
                    # emits staged by tile t-1 become poppable now: their
                    # att8 prep races only the first few j-steps (gated)
                    pending.extend(staged)
                    staged.clear()
                    if t < NI - 2:
                        seg = (
                            rope_units(t + 1)
                            + _qk4(base + 2 * t + 4)
                            + _qk4(base + 2 * t + 5)
                            + prefetch_units(base + 2 * t + 6)
                            + chunk_v_units(base + 2 * t + 2)
                            + chunk_v_units(base + 2 * t + 3)
                        )
                    elif t == NI - 2:
                        seg = (
                            rope_units(t + 1)
                            + prefetch_units(base + 2 * t + 6)
                            + chunk_v_units(base + 2 * t + 2)
                            + chunk_v_units(base + 2 * t + 3)
                        )
                    else:
                        seg = []
                    final_tile = b == B - 1 and t == NI - 1
                    # hold pops on every batch's last tile: its seg is empty,
                    # so saved emits are the only PE filler for the drain
                    # windows and the following prologue/tail boundary
                    batch_last = t == NI - 1
                    for h in range(HPC):
                        att_instance(
                            b, h, t, seg,
                            last=(final_tile and h == HPC - 1),
                            hold=(h + 1 if batch_last else 0),
                        )
                    staged.extend(
                        (b, t2, dd)
                        for t2 in range(4 * t, 4 * t + 4)
                        for dd in range(4)
                    )
                    while seg:
                        seg.pop(0)()
            pending.extend(staged)
            staged.clear()
            while pending:
                emit_y(*pending.pop(0))
    nc.compile()
    return nc


_NC = None


def _get_nc():
    global _NC
    if _NC is None:
        _NC = build_nc()
    return _NC


def _fp8_split(a, s):
    import ml_dtypes

    f8 = ml_dtypes.float8_e4m3fn
    hi = (a * np.float32(s)).astype(f8)
    lo = ((a * np.float32(s)) - hi.astype(np.float32)).astype(f8)
    return hi, lo


def _host_inputs(x, mask, wq, wk, wv, wo):
    import ml_dtypes

    x = np.asarray(x, np.float32)
    wq = np.asarray(wq, np.float32)
    wk = np.asarray(wk, np.float32)
    wv = np.asarray(wv, np.float32)
    wo = np.asarray(wo, np.float32)

    # x chunks: [chunk, p, c, hi(256)|lo(256)] fp8, d = 128c + p
    xr = (
        x.transpose(0, 2, 1)          # [B, D, L]
        .reshape(B, 16, 128, NT256, 256)
        .transpose(0, 3, 2, 1, 4)     # [b, tt, p, c, tok]
    )
    xh, xl = _fp8_split(xr, SX)
    xq = np.ascontiguousarray(
        np.concatenate([xh, xl], axis=-1).reshape(NCHUNK, 128, 16, 512)
    )

    # permute head dims so RoPE pairs are (i, i+32): [evens, odds, pass]
    perm128 = np.concatenate(
        [np.arange(0, ROPE, 2), np.arange(1, ROPE, 2), np.arange(ROPE, HD)]
    )
    permD = np.concatenate([h * HD + perm128 for h in range(H)])
    wq_p = (wq * np.float32(1.0 / np.sqrt(HD)))[:, permD]
    wk_p = wk[:, permD]

    def wtile(w2d):  # [D, DQ] -> [128 p, 16 c, DQ]
        return np.ascontiguousarray(
            w2d.reshape(16, 128, DQ).transpose(1, 0, 2)
        )

    # RoPE tables, matching reference fp32 math (dim=64, repeat-2 interleave)
    ts_ = np.arange(0, ROPE, 2, dtype=np.float32)
    inv = (np.float32(10000.0) ** (-ts_ / np.float32(ROPE))).astype(np.float32)
    grid = np.arange(L, dtype=np.float32)[:, None] * inv[None, :]  # [L, 32]
    ccm = np.empty((ROPE, L), np.float32)
    ccm[0:32] = ccm[32:64] = np.cos(grid).T
    ssm = np.empty((ROPE, L), np.float32)
    ssm[0:32] = -np.sin(grid).T
    ssm[32:64] = np.sin(grid).T
    ccm = ccm.astype(ml_dtypes.bfloat16)
    ssm = ssm.astype(ml_dtypes.bfloat16)

    # diagonal-quarter masks for ST tiles: m4[j, q, i] = mask[i, 128q + j]
    mm = np.asarray(mask[0, 0, :512, :512])
    m4 = (
        mm.T.reshape(4, 128, 512).transpose(1, 0, 2).astype(ml_dtypes.bfloat16)
    )  # [j, q, i]

    in_maps = []
    for c in range(NCORES):
        sl = slice(DQ * c, DQ * c + DQ)
        def pack_hl(w2d, s):
            hi, lo = _fp8_split(wtile(w2d), s)
            return np.ascontiguousarray(np.concatenate([hi, lo], axis=-1))

        wq8_ = pack_hl(wq_p[:, sl], SWQ)
        wk8_ = pack_hl(wk_p[:, sl], SWK)
        wv8_ = pack_hl(wv[:, sl], SWV)
        wo_t = np.ascontiguousarray(
            wo[sl, :].reshape(HPC, HD, D).transpose(1, 0, 2)
        ).astype(ml_dtypes.bfloat16)
        in_maps.append(
            {
                "xq": xq,
                "wq8": wq8_, "wk8": wk8_, "wv8": wv8_,
                "wo": wo_t,
                "cc": ccm,
                "ss": ssm,
                "m4": m4,
            }
        )
    return in_maps


def _reference_host(x, mask, wq, wk, wv, wo):
    """Exact-math numpy fallback (used only if the mask is not causal-tril)."""
    Hh, P = H, 64
    xx = np.asarray(x, np.float32)
    Bb, Ll, Dd = xx.shape
    K = Dd // Hh

    def rope(t):  # [b,h,s,d]
        d, s = t.shape[-1], t.shape[-2]
        ts_ = np.arange(0, d, 2, dtype=np.float32)
        inv = (np.float32(10000.0) ** (-ts_ / np.float32(d)))
        grid = np.arange(s, dtype=np.float32)[:, None] * inv[None, :]
        sin = np.repeat(np.sin(grid), 2, axis=-1)[None, None]
        cos = np.repeat(np.cos(grid), 2, axis=-1)[None, None]
        x1, x2 = t[..., ::2], t[..., 1::2]
        xs = np.stack([-x2, x1], axis=-1).reshape(t.shape)
        return t * cos + xs * sin

    def split(t):
        return t.reshape(Bb, Ll, Hh, K).transpose(0, 2, 1, 3)

    q = split(xx @ np.asarray(wq, np.float32)) / np.sqrt(K)
    q = np.concatenate([rope(q[..., :P]), q[..., P:]], axis=-1)
    k = split(xx @ np.asarray(wk, np.float32))
    k = np.concatenate([rope(k[..., :P]), k[..., P:]], axis=-1)
    v = split(xx @ np.asarray(wv, np.float32))
    s = np.einsum("bhik,bhjk->bhij", q, k)
    s = np.where(np.asarray(mask), s, np.float32(-1e8))
    s -= s.max(axis=-1, keepdims=True)
    e = np.exp(s)
    a = e / e.sum(axis=-1, keepdims=True)
    yy = np.einsum("bhij,bhjv->bhiv", a, v)
    yy = yy.transpose(0, 2, 1, 3).reshape(Bb, Ll, Dd)
    return (yy @ np.asarray(wo, np.float32)).astype(np.float32)


def kernel(**inputs):
    mask_arr = np.asarray(inputs["mask"])
    if not bool(
        (mask_arr[0, 0] == np.tril(np.ones((L, L), bool))).all()
    ):
        return _reference_host(
            inputs["x"], inputs["mask"], inputs["wq"], inputs["wk"],
            inputs["wv"], inputs["wo"],
        )
    nc = _get_nc()
    in_maps = _host_inputs(
        inputs["x"], inputs["mask"], inputs["wq"], inputs["wk"],
        inputs["wv"], inputs["wo"],
    )
    res = run_bass_kernel_spmd(nc, in_maps, core_ids=list(range(NCORES)))
    out = res.results[0]["y"].astype(np.float64)
    for c in range(1, NCORES):
        out += res.results[c]["y"]
    return out.astype(np.float32)
